# revision 5
# baseline (speedup 1.0000x reference)
"""Two-layer GAT on 8 Trainium2 NeuronCores.

Strategy: collective-minimal, SPMD over destination ranges.
- Layer-1 GEMM (x@w1) is REPLICATED: every core computes h for all N nodes
  and writes gather-ready packed rows [h | a_s] straight into its own DRAM.
  A per-core rotation of the node-row space keeps the SPMD program identical
  while placing each core's own destination windows at rows 0..NPAD-1.
- Edges (with self-loops) are sorted by destination; core c owns dst range
  [c*NPC, (c+1)*NPC) and computes those output rows entirely locally.
- Gathers use 1024-index chunks round-robined over 4 SWDGE queues: a single
  qPoolDynamic ring drains ~8.6us per 1024 descriptors regardless of element
  size, and the rings drain independently, so 4 queues give ~2.2us/chunk.
  (GpSimd desc-gen itself is only ~1.4us/chunk and is not the serializer.)
- Layer-2 rows are built DURING MP1 as 256B-padded fp8 gather rows held in
  SBUF: [h2(40) | 1.0 | pad | a_s2 replicated x42 | pad]. The 1.0 column
  makes the softmax normalizer fall out of the scatter matmul for free; the
  a_s2 replication (Act per-partition-bias broadcast) turns MP2's score adds
  and alpha scaling into fully PACKED 42-wide DVE ops -- the old per-edge
  strided column picks cost ~700ns/element on DVE slow mode (~210us total).
- The layer-2 AllGather ships the padded fp8 rows in 4 chunks of 5 windows,
  each fired as soon as its windows finish, overlapping MP1; MP2 gathers
  fp8 directly (transpose=False gathers are byte movers; the old bf16
  expand chain cost ~140us of tiny strided DMA descriptors).
- One-hot scatter (Swin) / gather (STw) matrices are precomputed on the
  HOST and shipped as bf16: STw stays resident in SBUF for MP1 and also
  distributes layer-2's a_d per edge (adew2all), so MP2 needs no STw.
- Features are head-interleaved (f = c*H + h) so per-edge message scaling
  broadcasts alpha along a packed 8-wide run (fast DVE mode).
- Act engine: Prelu/Exp/Relu from one table; the final log(sum) runs ONCE
  over all windows after the loop.
"""
import sys

sys.path.insert(0, "/opt/trn_rl_repo")
import numpy as np
import ml_dtypes

import concourse.bass as bass
import concourse.bacc as bacc
import concourse.mybir as mybir
import concourse.tile as tile
from concourse.bass_utils import run_bass_kernel_spmd
from concourse.masks import make_identity

BF16 = mybir.dt.bfloat16
F32 = mybir.dt.float32
I16 = mybir.dt.int16
FP8 = mybir.dt.float8e4
nbf16 = ml_dtypes.bfloat16
AF = mybir.ActivationFunctionType

NCORES = 8
NEG = 0.2
P = 128
GC = 8          # edge tiles per gather chunk (1024 idxs = SWDGE ring cap)
NSQ = 4         # SWDGE queues (hardware max)
CHW = 5         # windows per AllGather chunk
PR = 256        # padded fp8 gather row bytes for layer 2
PC = 96         # compact fp8 row width shipped through the AllGather
CW = 96         # fp8->bf16 cast width (covers h2|1|pad|a_s2 rep)
AS0, AS1 = 48, 90   # a_s2 replicated at fp8 cols [48, 90)


def _to_bf(a):
    return np.ascontiguousarray(np.asarray(a, dtype=np.float32).astype(nbf16))


def _build_program(IN_C, F1, H, C, OUT_C, NPAD, T, TPW, W, TG, NROWS):
    KT1 = IN_C // P            # 4 k-tiles for GEMM1
    KT2 = F1 // P              # 2 k-tiles for GEMM2
    F1E = F1 + 2 * H           # gemm1 columns: [w1 | w1@As | w1@Ad]
    F1A = F1 + H               # packed row content: [h | a_s]
    F1R = ((F1A * 2 + 255) // 256) * 128   # L1 gather row width (bf16)
    OCE = OUT_C + 2            # gemm2 columns: [w2 | w2@as2 | w2@ad2]
    OC1 = OUT_C + 1            # layer-2 scatter rhs width consumed: [msg | exp]
    W42 = OUT_C + 2            # packed 42-wide lane for L2 score/alpha ops
    MT = NROWS // P            # 160 m-tiles in replicated GEMM1
    SLAB = 8                   # m-tiles per xT slab
    NSLAB = MT // SLAB
    WB = 4                     # m-tiles per hfull write
    NCH = TG // GC             # gather chunks
    NI = GC * P                # idxs per gather
    IW = NI // 16              # idx cols per chunk
    NQC = W // CHW             # AllGather chunks
    CR = CHW * P               # rows per AllGather chunk (per core)

    nc = bacc.Bacc(num_devices=NCORES, num_swdge_queues=NSQ)

    xT_d = nc.dram_tensor("xT", [IN_C, NROWS], BF16, kind="ExternalInput")
    w1e_d = nc.dram_tensor("w1e", [IN_C, F1E], BF16, kind="ExternalInput")
    w2e_d = nc.dram_tensor("w2e", [F1, OCE], BF16, kind="ExternalInput")
    idx1_d = nc.dram_tensor("idx1", [P, NCH * IW], I16, kind="ExternalInput")
    idx2_d = nc.dram_tensor("idx2", [P, NCH * IW], I16, kind="ExternalInput")
    swin_d = nc.dram_tensor("swin", [P, T * P], BF16, kind="ExternalInput")
    stw_d = nc.dram_tensor("stw", [P, T * P], BF16, kind="ExternalInput")
    out_d = nc.dram_tensor("out", [NPAD, OUT_C], F32, kind="ExternalOutput")

    hfull_d = nc.dram_tensor("hfull", [NROWS, F1R], BF16)
    h2loc_d = nc.dram_tensor("h2loc", [NPAD, PC], FP8)
    h2pkc_d = nc.dram_tensor("h2pkc", [NROWS, PC], FP8, addr_space="Shared")
    h2pk_d = nc.dram_tensor("h2pk", [NROWS, PR], FP8)

    rg = [list(range(NCORES))]

    with tile.TileContext(nc) as tc:
        with (
            tc.tile_pool(name="const", bufs=1) as cp,
            tc.tile_pool(name="persist", bufs=1) as pp,
        ):
            ident = cp.tile([P, P], BF16)
            make_identity(nc, ident[:])
            w1sb = cp.tile([P, KT1 * F1E], BF16)
            for k in range(KT1):
                nc.sync.dma_start(out=w1sb[:, k * F1E:(k + 1) * F1E], in_=w1e_d[k * P:(k + 1) * P, :])
            w2sb = cp.tile([P, KT2 * OCE], BF16)
            for k in range(KT2):
                nc.sync.dma_start(out=w2sb[:, k * OCE:(k + 1) * OCE], in_=w2e_d[k * P:(k + 1) * P, :])
            idx1sb = cp.tile([P, NCH * IW], I16)
            nc.gpsimd.dma_start(out=idx1sb[:], in_=idx1_d[:, :])
            idx2sb = cp.tile([P, NCH * IW], I16)
            zeros42 = cp.tile([P, W42], F32)
            nc.vector.memset(zeros42[:], 0.0)
            stwp_cm = tc.tile_pool(name="stwp", bufs=2)
            stwp = stwp_cm.__enter__()
            stwq_t = {}                        # rolling stw quarters (one per CHW windows)
            SQC = CHW * TPW * P                # stw cols per quarter
            QS = (T * P) // 4

            adw = pp.tile([P, W * H], BF16)    # a_d for local windows
            ad2w = pp.tile([P, W], BF16)       # layer-2 a_d for local windows
            adew2all = pp.tile([P, T], BF16)   # layer-2 a_d per edge (from MP1)
            h2all = pp.tile([P, W * PC], FP8)  # compact L2 gather rows (fp8)
            o2all = pp.tile([P, W * OUT_C], F32)
            mnall = pp.tile([P, W], F32)
            ssall = pp.tile([P, W], F32)

            # ---------------- Phase A: replicated GEMM1 -> hfull ----------------
            with (
                tc.tile_pool(name="slab", bufs=4) as slab,
                tc.tile_pool(name="psA", bufs=7, space="PSUM") as psA,
                tc.tile_pool(name="stg", bufs=4) as stg,
            ):
                for g in range(NSLAB):
                    xs = slab.tile([P, KT1 * SLAB * P], BF16, tag="xs")
                    for k in range(KT1):
                        nc.sync.dma_start(
                            out=xs[:, k * SLAB * P:(k + 1) * SLAB * P],
                            in_=xT_d[k * P:(k + 1) * P, g * SLAB * P:(g + 1) * SLAB * P],
                        )
                    for mm in range(SLAB):
                        m = g * SLAB + mm
                        ph = psA.tile([P, F1E], F32, tag="ph")
                        for k in range(KT1):
                            nc.tensor.matmul(
                                ph[:],
                                lhsT=xs[:, k * SLAB * P + mm * P: k * SLAB * P + (mm + 1) * P],
                                rhs=w1sb[:, k * F1E:(k + 1) * F1E],
                                start=(k == 0), stop=(k == KT1 - 1),
                            )
                        if mm % WB == 0:
                            hb = stg.tile([P, WB * F1A], BF16, tag="hb")
                        dst = hb[:, (mm % WB) * F1A:(mm % WB + 1) * F1A]
                        if mm % 2 == 0:
                            nc.scalar.activation(dst, ph[:, :F1A], AF.Copy)
                        else:
                            nc.vector.tensor_copy(out=dst, in_=ph[:, :F1A])
                        if m < W:
                            nc.vector.tensor_copy(out=adw[:, m * H:(m + 1) * H], in_=ph[:, F1 + H:F1E])
                        if mm % WB == WB - 1:
                            g8 = m // WB
                            nc.gpsimd.dma_start(
                                out=hfull_d[g8 * WB * P:(g8 + 1) * WB * P, :F1A]
                                    .rearrange("(t p) c -> p t c", p=P),
                                in_=hb[:].rearrange("p (t c) -> p t c", c=F1A),
                            )

            # idx2 load on SP after all xs loads (in-order queue
            # prevents the scheduler hoisting it into the startup window)
            nc.sync.dma_start(out=idx2sb[:], in_=idx2_d[:, :])

            # ---------------- Phase B: L1 message passing + GEMM2 ----------------
            with (
                tc.tile_pool(name="gp", bufs=8) as gp,
                tc.tile_pool(name="swp", bufs=3) as swp,
                tc.tile_pool(name="psAcc", bufs=3, space="PSUM") as psAcc,
                tc.tile_pool(name="psAde", bufs=3, space="PSUM") as psAde,
                tc.tile_pool(name="psT", bufs=1, space="PSUM") as psT,
                tc.tile_pool(name="psC", bufs=1, space="PSUM") as psC,
                tc.tile_pool(name="wp", bufs=2) as wp,
                tc.tile_pool(name="wps", bufs=3) as wps,
            ):
                chunk_tiles = {}

                def get_chunk(cc):
                    if cc in chunk_tiles:
                        return chunk_tiles[cc]
                    gt = gp.tile([P, GC * F1R], BF16, tag="gath")
                    nc.gpsimd.dma_gather(
                        out_ap=gt[:].rearrange("p (t f) -> p t f", f=F1R),
                        in_ap=hfull_d.ap(),
                        idxs_ap=idx1sb[:, cc * IW:(cc + 1) * IW],
                        num_idxs=NI, num_idxs_reg=NI, elem_size=F1R,
                        queue_num=cc % NSQ,
                    )
                    chunk_tiles[cc] = gt
                    return gt

                def segments(w):
                    """[(chunk, slot0, slot1, tl0)] covering tiles of window w."""
                    segs = []
                    t0, t1 = w * TPW, (w + 1) * TPW
                    t = t0
                    while t < t1:
                        cc = t // GC
                        s0 = t - cc * GC
                        s1 = min(GC, t1 - cc * GC)
                        segs.append((cc, s0, s1, t - t0))
                        t = cc * GC + s1
                    return segs

                for w in range(W):
                    if w % CHW == 0:        # stream stw quarter for these windows
                        q = w // CHW
                        stq = stwp.tile([P, SQC], BF16, tag="stwq")
                        nc.sync.dma_start(out=stq[:], in_=stw_d[:, q * SQC:(q + 1) * SQC])
                        stwq_t[q] = stq
                    if w >= 6 and (w - 6) % CHW == 0:
                        # fire AllGather chunk (w-6)//CHW: its h2loc store
                        # landed ~1.5 windows ago, so the gpsimd queue
                        # does not stall on the wait
                        q = (w - 6) // CHW
                        nc.gpsimd.collective_compute(
                            "AllGather", mybir.AluOpType.bypass, replica_groups=rg,
                            ins=[h2loc_d[q * CR:(q + 1) * CR, :]],
                            outs=[h2pkc_d[q * CR * NCORES:(q + 1) * CR * NCORES, :]],
                        )
                    swin = swp.tile([P, TPW * P], BF16, tag="swin")
                    nc.sync.dma_start(
                        out=swin[:], in_=swin_d[:, w * TPW * P:(w + 1) * TPW * P])
                    segs = segments(w)
                    for (cc, s0, s1, tl0) in segs:
                        get_chunk(cc)
                    stq = stwq_t[w // CHW]
                    tb = (w % CHW) * TPW * P
                    adew = psAde.tile([P, TPW * H + TPW], F32, tag="adew")
                    for tl in range(TPW):
                        nc.tensor.matmul(
                            adew[:, tl * H:(tl + 1) * H],
                            lhsT=stq[:, tb + tl * P:tb + (tl + 1) * P],
                            rhs=adw[:, w * H:(w + 1) * H],
                            start=True, stop=True,
                        )
                    # scores: a_s read straight from the gather chunks;
                    # adew staged to SBUF bf16 (Act) so the adds run packed
                    adsb = wps.tile([P, TPW * H], BF16, tag="adsb")
                    nc.scalar.activation(adsb[:], adew[:, :TPW * H], AF.Copy)
                    escw = wps.tile([P, TPW * H], BF16, tag="escw")
                    for (cc, s0, s1, tl0) in segs:
                        gt = chunk_tiles[cc]
                        nc.vector.tensor_add(
                            out=escw[:, tl0 * H:(tl0 + s1 - s0) * H]
                                .rearrange("p (t h) -> p t h", h=H),
                            in0=adsb[:, tl0 * H:(tl0 + s1 - s0) * H]
                                .rearrange("p (t h) -> p t h", h=H),
                            in1=gt[:].rearrange("p (t f) -> p t f", f=F1R)[:, s0:s1, F1:F1A],
                        )
                    lrw = wps.tile([P, TPW * H], BF16, tag="lrw")
                    nc.scalar.activation(lrw[:], escw[:], AF.Prelu, alpha=NEG)
                    expw = wps.tile([P, TPW * H], BF16, tag="expw")
                    nc.scalar.activation(expw[:], lrw[:], AF.Exp)
                    # rhs = [h * alpha (head-interleaved) | alpha] per tile
                    rhs = wp.tile([P, TPW * F1A], BF16, tag="rhs")
                    for (cc, s0, s1, tl0) in segs:
                        gt = chunk_tiles[cc]
                        nt = s1 - s0
                        nc.vector.tensor_mul(
                            out=rhs[:].rearrange("p (t f) -> p t f", f=F1A)[:, tl0:tl0 + nt, :F1]
                                .rearrange("p t (c h) -> p t c h", h=H),
                            in0=gt[:].rearrange("p (t f) -> p t f", f=F1R)[:, s0:s1, :F1]
                                .rearrange("p t (c h) -> p t c h", h=H),
                            in1=expw[:].rearrange("p (t h) -> p t h", h=H)[:, tl0:tl0 + nt, :]
                                .rearrange("p t (o h) -> p t o h", o=1)
                                .to_broadcast([P, nt, C, H]),
                        )
                    nc.scalar.activation(
                        rhs[:].rearrange("p (t f) -> p t f", f=F1A)[:, :, F1:F1A],
                        expw[:].rearrange("p (t h) -> p t h", h=H), AF.Copy)
                    acc = psAcc.tile([P, F1A], F32, tag="acc")
                    for tl in range(TPW):
                        nc.tensor.matmul(
                            acc[:], lhsT=swin[:, tl * P:(tl + 1) * P],
                            rhs=rhs[:, tl * F1A:(tl + 1) * F1A],
                            start=(tl == 0), stop=(tl == TPW - 1),
                        )
                    # finalize: o1 = acc/z, ELU, transpose, GEMM2
                    zs = wps.tile([P, H], F32, tag="zs")
                    nc.vector.tensor_scalar_add(out=zs[:], in0=acc[:, F1:F1A], scalar1=1e-16)
                    zr = wps.tile([P, H], F32, tag="zr")
                    nc.vector.reciprocal(zr[:], zs[:])
                    o1 = wps.tile([P, F1], F32, tag="o1")
                    nc.vector.tensor_mul(
                        out=o1[:].rearrange("p (c h) -> p c h", h=H),
                        in0=acc[:, :F1].rearrange("p (c h) -> p c h", h=H),
                        in1=zr[:].rearrange("p (o h) -> p o h", o=1).to_broadcast([P, C, H]),
                    )
                    rneg = wps.tile([P, F1], F32, tag="rneg")
                    nc.scalar.activation(rneg[:], o1[:], AF.Relu, scale=-1.0)
                    em = wps.tile([P, F1], F32, tag="em")
                    nc.scalar.activation(em[:], rneg[:], AF.Exp, scale=-1.0)
                    mx = wps.tile([P, F1], F32, tag="mx")
                    nc.scalar.activation(mx[:], o1[:], AF.Relu)
                    h1 = wps.tile([P, F1], BF16, tag="h1")
                    nc.vector.scalar_tensor_tensor(
                        out=h1[:], in0=em[:], scalar=-1.0, in1=mx[:],
                        op0=mybir.AluOpType.add, op1=mybir.AluOpType.add,
                    )
                    h1Tw = wps.tile([P, KT2 * P], BF16, tag="h1Tw")
                    for fc in range(KT2):
                        tp = psT.tile([P, P], BF16, tag="tp")
                        nc.tensor.transpose(tp[:], h1[:, fc * P:(fc + 1) * P], ident[:])
                        nc.scalar.activation(h1Tw[:, fc * P:(fc + 1) * P], tp[:], AF.Copy)
                    p2 = psC.tile([P, OCE], F32, tag="p2")
                    for k in range(KT2):
                        nc.tensor.matmul(
                            p2[:],
                            lhsT=h1Tw[:, k * P:(k + 1) * P],
                            rhs=w2sb[:, k * OCE:(k + 1) * OCE],
                            start=(k == 0), stop=(k == KT2 - 1),
                        )
                    # build the padded fp8 L2 gather row for this window:
                    # [h2(40) | 1.0 | pad | a_s2 x42 | pad]
                    nc.scalar.activation(
                        h2all[:, w * PC:w * PC + OUT_C], p2[:, :OUT_C], AF.Copy)
                    nc.vector.memset(h2all[:, w * PC + OUT_C:w * PC + OUT_C + 1], 1.0)
                    as2sb = wps.tile([P, 1], F32, tag="as2sb")
                    nc.vector.tensor_copy(out=as2sb[:], in_=p2[:, OUT_C:OUT_C + 1])
                    nc.scalar.activation(
                        h2all[:, w * PC + AS0:w * PC + AS1], zeros42[:],
                        AF.Identity, bias=as2sb[:])
                    nc.vector.tensor_copy(out=ad2w[:, w:w + 1], in_=p2[:, OCE - 1:OCE])
                    # layer-2 a_d per edge, computed now while stw tile is hot
                    for tl in range(TPW):
                        nc.tensor.matmul(
                            adew[:, TPW * H + tl:TPW * H + tl + 1],
                            lhsT=stq[:, tb + tl * P:tb + (tl + 1) * P],
                            rhs=ad2w[:, w:w + 1],
                            start=True, stop=True,
                        )
                    nc.scalar.activation(
                        adew2all[:, w * TPW:(w + 1) * TPW], adew[:, TPW * H:], AF.Copy)
                    # ship finished AllGather chunk inputs while MP1 continues
                    if w % CHW == CHW - 1:
                        q = w // CHW
                        nc.sync.dma_start(
                            out=h2loc_d[q * CR:(q + 1) * CR, :]
                                .rearrange("(t p) c -> p t c", p=P),
                            in_=h2all[:, q * CHW * PC:(q + 1) * CHW * PC]
                                .rearrange("p (t c) -> p t c", c=PC),
                        )
                # last AllGather chunk fires right after its store
                q = W // CHW - 1
                nc.gpsimd.collective_compute(
                    "AllGather", mybir.AluOpType.bypass, replica_groups=rg,
                    ins=[h2loc_d[q * CR:(q + 1) * CR, :]],
                    outs=[h2pkc_d[q * CR * NCORES:(q + 1) * CR * NCORES, :]],
                )

            stwp_cm.__exit__(None, None, None)   # stw quarters dead
            swall_cm = tc.tile_pool(name="swall", bufs=1)
            swallp = swall_cm.__enter__()
            swall = swallp.tile([P, T * P], BF16)
            for q in range(4):
                nc.sync.dma_start(out=swall[:, q * QS:(q + 1) * QS],
                                  in_=swin_d[:, q * QS:(q + 1) * QS])
            # expand compact 96B AllGather rows to 256B-padded gather rows
            CRN = CR * NCORES // P    # chunk rows per partition
            with tc.tile_pool(name="xp", bufs=2) as xp:
                for q in range(W // CHW):
                    e96 = xp.tile([P, CRN * PC], FP8, tag="e96")
                    nc.gpsimd.dma_start(
                        out=e96[:].rearrange("p (t c) -> p t c", c=PC),
                        in_=h2pkc_d[q * CR * NCORES:(q + 1) * CR * NCORES, :]
                            .rearrange("(p t) c -> p t c", p=P))
                    e256 = xp.tile([P, CRN * PR], FP8, tag="e256")
                    nc.vector.tensor_copy(
                        out=e256[:].rearrange("p (t c) -> p t c", c=PR)[:, :, :PC],
                        in_=e96[:].rearrange("p (t c) -> p t c", c=PC))
                    nc.sync.dma_start(
                        out=h2pk_d[q * CR * NCORES:(q + 1) * CR * NCORES, :]
                            .rearrange("(p t) c -> p t c", p=P),
                        in_=e256[:].rearrange("p (t c) -> p t c", c=PR))

            # ---------------- Phase D: layer-2 message passing ----------------
            with (
                tc.tile_pool(name="gp2", bufs=10) as gp2,
                tc.tile_pool(name="gbp", bufs=10) as gbp,
                tc.tile_pool(name="psAcc2", bufs=2, space="PSUM") as psAcc2,
                tc.tile_pool(name="wp2", bufs=2) as wp2,
                tc.tile_pool(name="wps2", bufs=3) as wps2,
                tc.tile_pool(name="outp", bufs=2) as outp,
            ):
                chunk2 = {}

                def get_chunk2(cc):
                    if cc in chunk2:
                        return chunk2[cc]
                    g2 = gp2.tile([P, GC * PR], FP8, tag="gath2")
                    nc.gpsimd.dma_gather(
                        out_ap=g2[:].rearrange("p (t f) -> p t f", f=PR),
                        in_ap=h2pk_d.ap(),
                        idxs_ap=idx2sb[:, cc * IW:(cc + 1) * IW],
                        num_idxs=NI, num_idxs_reg=NI, elem_size=PR,
                        queue_num=cc % NSQ,
                    )
                    gb = gbp.tile([P, GC * CW], BF16, tag="gbf")
                    if cc % 2:
                        nc.vector.tensor_copy(
                            out=gb[:].rearrange("p (t f) -> p t f", f=CW),
                            in_=g2[:].rearrange("p (t f) -> p t f", f=PR)[:, :, :CW])
                    else:
                        nc.scalar.activation(
                            gb[:].rearrange("p (t f) -> p t f", f=CW),
                            g2[:].rearrange("p (t f) -> p t f", f=PR)[:, :, :CW], AF.Copy)
                    chunk2[cc] = gb
                    return gb

                def segments2(w):
                    segs = []
                    t0, t1 = w * TPW, (w + 1) * TPW
                    t = t0
                    while t < t1:
                        cc = t // GC
                        s0 = t - cc * GC
                        s1 = min(GC, t1 - cc * GC)
                        segs.append((cc, s0, s1, t - t0))
                        t = cc * GC + s1
                    return segs

                for w in range(W):
                    swin2 = swall[:, w * TPW * P:(w + 1) * TPW * P]
                    segs = segments2(w)
                    for (cc, s0, s1, tl0) in segs:
                        get_chunk2(cc)
                    # packed 42-wide: esc = a_s2(rep) + a_d2(bcast); then
                    # Prelu/Exp give alpha replicated x42 with no picks
                    esc2w = wps2.tile([P, TPW * W42], BF16, tag="esc2w")
                    for (cc, s0, s1, tl0) in segs:
                        gb = chunk2[cc]
                        nt = s1 - s0
                        nc.vector.tensor_add(
                            out=esc2w[:].rearrange("p (t f) -> p t f", f=W42)[:, tl0:tl0 + nt, :],
                            in0=gb[:].rearrange("p (t f) -> p t f", f=CW)[:, s0:s1, AS0:AS1],
                            in1=adew2all[:, w * TPW + tl0:w * TPW + tl0 + nt]
                                .rearrange("p (t o) -> p t o", o=1)
                                .to_broadcast([P, nt, W42]),
                        )
                    lr2w = wps2.tile([P, TPW * W42], BF16, tag="lr2w")
                    nc.scalar.activation(lr2w[:], esc2w[:], AF.Prelu, alpha=NEG)
                    exp2w = wps2.tile([P, TPW * W42], BF16, tag="exp2w")
                    nc.scalar.activation(exp2w[:], lr2w[:], AF.Exp)
                    # rhs = [h2 | 1] * alpha, all packed 42-wide
                    rhs2 = wp2.tile([P, TPW * W42], BF16, tag="rhs2")
                    for (cc, s0, s1, tl0) in segs:
                        gb = chunk2[cc]
                        nt = s1 - s0
                        nc.vector.tensor_mul(
                            out=rhs2[:].rearrange("p (t f) -> p t f", f=W42)[:, tl0:tl0 + nt, :],
                            in0=gb[:].rearrange("p (t f) -> p t f", f=CW)[:, s0:s1, :W42],
                            in1=exp2w[:].rearrange("p (t f) -> p t f", f=W42)[:, tl0:tl0 + nt, :],
                        )
                    acc2 = psAcc2.tile([P, OC1], F32, tag="acc2")
                    for tl in range(TPW):
                        nc.tensor.matmul(
                            acc2[:], lhsT=swin2[:, tl * P:(tl + 1) * P],
                            rhs=rhs2[:, tl * W42:tl * W42 + OC1],
                            start=(tl == 0), stop=(tl == TPW - 1),
                        )
                    zs2 = wps2.tile([P, 1], F32, tag="zs2")
                    nc.vector.tensor_scalar_add(out=zs2[:], in0=acc2[:, OUT_C:OC1], scalar1=1e-16)
                    zr2 = wps2.tile([P, 1], F32, tag="zr2")
                    nc.vector.reciprocal(zr2[:], zs2[:])
                    nc.vector.tensor_mul(
                        out=o2all[:, w * OUT_C:(w + 1) * OUT_C], in0=acc2[:, :OUT_C],
                        in1=zr2[:].to_broadcast([P, OUT_C]),
                    )
                    nc.vector.tensor_reduce(
                        out=mnall[:, w:w + 1], in_=o2all[:, w * OUT_C:(w + 1) * OUT_C],
                        axis=mybir.AxisListType.X,
                        op=mybir.AluOpType.max, negate=True,
                    )
                    ex = wps2.tile([P, OUT_C], F32, tag="ex")
                    nc.scalar.activation(
                        ex[:], o2all[:, w * OUT_C:(w + 1) * OUT_C], AF.Exp,
                        bias=mnall[:, w:w + 1], accum_out=ssall[:, w:w + 1],
                    )

                # single Ln pass over all windows, then per-window bias add
                lnall = wps2.tile([P, W], F32, tag="lnall")
                nc.scalar.activation(lnall[:], ssall[:], AF.Ln)
                comb = wps2.tile([P, W], F32, tag="comb")
                nc.vector.tensor_sub(out=comb[:], in0=mnall[:], in1=lnall[:])
                for w in range(W):
                    if w % 4 == 0:
                        fin4 = outp.tile([P, 4 * OUT_C], F32, tag="fin4")
                    nc.scalar.activation(
                        fin4[:, (w % 4) * OUT_C:(w % 4 + 1) * OUT_C],
                        o2all[:, w * OUT_C:(w + 1) * OUT_C],
                        AF.Identity, bias=comb[:, w:w + 1])
                    if w % 4 == 3:
                        g4 = w // 4
                        nc.scalar.dma_start(
                            out=out_d[g4 * 4 * P:(g4 + 1) * 4 * P, :]
                                .rearrange("(t p) c -> p t c", p=P),
                            in_=fin4[:].rearrange("p (t c) -> p t c", c=OUT_C),
                        )
            swall_cm.__exit__(None, None, None)

    nc.compile()
    return nc


def _prepare(x, edge_index, w1, att_src1, att_dst1, b1, w2, att_src2, att_dst2, b2):
    x = np.asarray(x, dtype=np.float32)
    edge_index = np.asarray(edge_index)
    w1 = np.asarray(w1, dtype=np.float32)
    att_src1 = np.asarray(att_src1, dtype=np.float32)
    att_dst1 = np.asarray(att_dst1, dtype=np.float32)
    b1 = np.asarray(b1, dtype=np.float32)
    w2 = np.asarray(w2, dtype=np.float32)
    att_src2 = np.asarray(att_src2, dtype=np.float32)
    att_dst2 = np.asarray(att_dst2, dtype=np.float32)
    b2 = np.asarray(b2, dtype=np.float32)
    assert not np.any(b1) and not np.any(b2), "nonzero bias unsupported"

    N, IN_C = x.shape
    H, C = att_src1.shape
    F1 = H * C
    OUT_C = w2.shape[1]
    assert N % NCORES == 0
    NPC = N // NCORES
    W = (NPC + P - 1) // P
    NPAD = W * P
    NROWS = NCORES * NPAD
    assert NROWS < 32768
    assert W % CHW == 0

    # ---- edges: append self-loops, sort by destination ----
    src = np.concatenate([edge_index[0].astype(np.int64), np.arange(N, dtype=np.int64)])
    dst = np.concatenate([edge_index[1].astype(np.int64), np.arange(N, dtype=np.int64)])
    order = np.argsort(dst, kind="stable")
    src, dst = src[order], dst[order]

    core_of = dst // NPC
    bounds = np.searchsorted(dst, np.arange(NCORES + 1) * NPC)
    win_of = (dst - core_of * NPC) // P

    counts = np.zeros((NCORES, W), np.int64)
    for cidx in range(NCORES):
        w_arr = win_of[bounds[cidx]:bounds[cidx + 1]]
        counts[cidx] = np.bincount(w_arr, minlength=W)
    TPW = max(1, int(np.ceil(counts.max() / P)))
    T = W * TPW
    TG = ((T + GC - 1) // GC) * GC

    blocked = (src // NPC) * NPAD + (src % NPC)     # global padded row of src

    # layer-2 rows live in AllGather-chunked layout:
    # node (c, r): q = r // (CHW*P); row = q*CHW*P*NCORES + c*CHW*P + (r - q*CHW*P)
    CR = CHW * P
    src_c = src // NPC
    src_r = src % NPC
    src_q = src_r // CR
    blocked2 = src_q * CR * NCORES + src_c * CR + (src_r - src_q * CR)

    NI = GC * P
    IW = NI // 16
    NCH = TG // GC

    def pack_idx(ids):
        """gather-index layout: chunk cc's idxs at cols [cc*IW,(cc+1)*IW)."""
        idx16 = np.zeros((16, NCH * IW), np.int16)
        gpos = np.arange(TG * P)
        cc, ii = gpos // NI, gpos % NI
        full = np.zeros(TG * P, np.int16)
        full[:len(ids)] = ids
        idx16[ii % 16, cc * IW + ii // 16] = full
        return np.tile(idx16, (8, 1))

    in_maps = []
    xTf = np.zeros((IN_C, NROWS), np.float32)
    xTf = xTf.reshape(IN_C, NCORES, NPAD)
    xTf[:, :, :NPC] = x.T.reshape(IN_C, NCORES, NPC)
    xTf = xTf.reshape(IN_C, NROWS)
    xTf_bf = _to_bf(xTf)

    # head-interleaved permutation: new col c*H+h <- old col h*C+c
    f_old = np.arange(F1)
    h_idx, c_idx = f_old // C, f_old % C
    f_new = c_idx * H + h_idx
    perm = np.empty(F1, np.int64)
    perm[f_new] = f_old          # perm[new] = old

    Asrc = np.zeros((F1, H), np.float32)
    Adst = np.zeros((F1, H), np.float32)
    for h in range(H):
        Asrc[h * C:(h + 1) * C, h] = att_src1[h]
        Adst[h * C:(h + 1) * C, h] = att_dst1[h]
    w1P = w1[:, perm]
    w1e = np.concatenate([w1P, w1 @ Asrc, w1 @ Adst], axis=1)
    w2P = w2[perm, :]
    w2e = np.concatenate([w2P, w2P @ att_src2.T, w2P @ att_dst2.T], axis=1)
    w1e_bf = _to_bf(w1e)
    w2e_bf = _to_bf(w2e)

    nc = _build_program(IN_C, F1, H, C, OUT_C, NPAD, T, TPW, W, TG, NROWS)

    for cidx in range(NCORES):
        ids_g = np.zeros(T * P, np.int64)         # L1: global padded row per slot
        ids_g2 = np.zeros(T * P, np.int64)        # L2: chunked-AllGather row per slot
        dloc = np.full(T * P, 255, np.int64)      # pad -> no one-hot match
        s_c = blocked[bounds[cidx]:bounds[cidx + 1]]
        s2_c = blocked2[bounds[cidx]:bounds[cidx + 1]]
        w_c = win_of[bounds[cidx]:bounds[cidx + 1]]
        d_c = dst[bounds[cidx]:bounds[cidx + 1]] - cidx * NPC
        wb = np.searchsorted(w_c, np.arange(W + 1))
        for w in range(W):
            n = wb[w + 1] - wb[w]
            base = w * TPW * P
            ids_g[base:base + n] = s_c[wb[w]:wb[w + 1]]
            ids_g2[base:base + n] = s2_c[wb[w]:wb[w + 1]]
            dloc[base:base + n] = d_c[wb[w]:wb[w + 1]] - w * P
        ids_rot = (ids_g - cidx * NPAD) % NROWS   # L1 rows are core-rotated

        # one-hot scatter/gather matrices
        M = np.zeros((T * P, P), np.float32)
        real = dloc < P
        M[np.nonzero(real)[0], dloc[real]] = 1.0
        M3 = M.reshape(T, P, P)
        swin = _to_bf(np.ascontiguousarray(M3.transpose(1, 0, 2)).reshape(P, T * P))
        stw_m = _to_bf(np.ascontiguousarray(M3.transpose(2, 0, 1)).reshape(P, T * P))

        in_maps.append({
            "xT": np.ascontiguousarray(np.roll(xTf_bf, -cidx * NPAD, axis=1)) if cidx else xTf_bf,
            "w1e": w1e_bf,
            "w2e": w2e_bf,
            "idx1": pack_idx(ids_rot.astype(np.int16)),
            "idx2": pack_idx(ids_g2.astype(np.int16)),
            "swin": swin,
            "stw": stw_m,
        })
    return nc, in_maps, NPC


def kernel(_trace=False, **inputs):
    nc, in_maps, NPC = _prepare(**inputs)
    res = run_bass_kernel_spmd(nc, in_maps, core_ids=list(range(NCORES)), trace=_trace)
    out = np.concatenate([res.results[cidx]["out"][:NPC] for cidx in range(NCORES)], axis=0)
    kernel.last_exec_time_ns = res.exec_time_ns
    kernel.last_res = res
    return out.astype(np.float32)


# revision 7
# speedup vs baseline: 1.0184x; 1.0184x over previous
"""Two-layer GAT on 8 Trainium2 NeuronCores.

Strategy: collective-minimal, SPMD over destination ranges.
- Layer-1 GEMM (x@w1) is REPLICATED: every core computes h for all N nodes
  and writes gather-ready packed rows [h | a_s] straight into its own DRAM.
  A per-core rotation of the node-row space keeps the SPMD program identical
  while placing each core's own destination windows at rows 0..NPAD-1.
- Edges (with self-loops) are sorted by destination; core c owns dst range
  [c*NPC, (c+1)*NPC) and computes those output rows entirely locally.
- Gathers use 1024-index chunks round-robined over 4 SWDGE queues: a single
  qPoolDynamic ring drains ~8.6us per 1024 descriptors regardless of element
  size, and the rings drain independently, so 4 queues give ~2.2us/chunk.
  (GpSimd desc-gen itself is only ~1.4us/chunk and is not the serializer.)
- Layer-2 rows are built DURING MP1 as 256B-padded fp8 gather rows held in
  SBUF: [h2(40) | 1.0 | pad | a_s2 replicated x42 | pad]. The 1.0 column
  makes the softmax normalizer fall out of the scatter matmul for free; the
  a_s2 replication (Act per-partition-bias broadcast) turns MP2's score adds
  and alpha scaling into fully PACKED 42-wide DVE ops -- the old per-edge
  strided column picks cost ~700ns/element on DVE slow mode (~210us total).
- The layer-2 AllGather ships the padded fp8 rows in 4 chunks of 5 windows,
  each fired as soon as its windows finish, overlapping MP1; MP2 gathers
  fp8 directly (transpose=False gathers are byte movers; the old bf16
  expand chain cost ~140us of tiny strided DMA descriptors).
- One-hot scatter (Swin) / gather (STw) matrices are precomputed on the
  HOST and shipped as bf16: STw stays resident in SBUF for MP1 and also
  distributes layer-2's a_d per edge (adew2all), so MP2 needs no STw.
- Features are head-interleaved (f = c*H + h) so per-edge message scaling
  broadcasts alpha along a packed 8-wide run (fast DVE mode).
- Act engine: Prelu/Exp/Relu from one table; the final log(sum) runs ONCE
  over all windows after the loop.
"""
import sys

sys.path.insert(0, "/opt/trn_rl_repo")
import numpy as np
import ml_dtypes

import concourse.bass as bass
import concourse.bacc as bacc
import concourse.mybir as mybir
import concourse.tile as tile
from concourse.bass_utils import run_bass_kernel_spmd
from concourse.masks import make_identity

BF16 = mybir.dt.bfloat16
F32 = mybir.dt.float32
I16 = mybir.dt.int16
FP8 = mybir.dt.float8e4
nbf16 = ml_dtypes.bfloat16
AF = mybir.ActivationFunctionType

NCORES = 8
NEG = 0.2
P = 128
GC = 8          # edge tiles per gather chunk (1024 idxs = SWDGE ring cap)
NSQ = 4         # SWDGE queues (hardware max)
CHW = 5         # windows per AllGather chunk
PR = 256        # padded fp8 gather row bytes for layer 2
PC = 96         # compact fp8 row width shipped through the AllGather
CW = 96         # fp8->bf16 cast width (covers h2|1|pad|a_s2 rep)
AS0, AS1 = 48, 90   # a_s2 replicated at fp8 cols [48, 90)


def _to_bf(a):
    return np.ascontiguousarray(np.asarray(a, dtype=np.float32).astype(nbf16))


def _build_program(IN_C, F1, H, C, OUT_C, NPAD, T, TPW, W, TG, NROWS):
    KT1 = IN_C // P            # 4 k-tiles for GEMM1
    KT2 = F1 // P              # 2 k-tiles for GEMM2
    F1E = F1 + 2 * H           # gemm1 columns: [w1 | w1@As | w1@Ad]
    F1A = F1 + H               # packed row content: [h | a_s]
    F1R = ((F1A * 2 + 255) // 256) * 128   # L1 gather row width (bf16)
    OCE = OUT_C + 2            # gemm2 columns: [w2 | w2@as2 | w2@ad2]
    OC1 = OUT_C + 1            # layer-2 scatter rhs width consumed: [msg | exp]
    W42 = OUT_C + 2            # packed 42-wide lane for L2 score/alpha ops
    MT = NROWS // P            # 160 m-tiles in replicated GEMM1
    SLAB = 8                   # m-tiles per xT slab
    NSLAB = MT // SLAB
    WB = 4                     # m-tiles per hfull write
    NCH = TG // GC             # gather chunks
    NI = GC * P                # idxs per gather
    IW = NI // 16              # idx cols per chunk
    NQC = W // CHW             # AllGather chunks
    CR = CHW * P               # rows per AllGather chunk (per core)
    WA = W - 6                 # windows in the early (hidden) AllGather chunk

    nc = bacc.Bacc(num_devices=NCORES, num_swdge_queues=NSQ)

    xT_d = nc.dram_tensor("xT", [IN_C, NROWS], BF16, kind="ExternalInput")
    w1e_d = nc.dram_tensor("w1e", [IN_C, F1E], BF16, kind="ExternalInput")
    w2e_d = nc.dram_tensor("w2e", [F1, OCE], BF16, kind="ExternalInput")
    idx1_d = nc.dram_tensor("idx1", [P, NCH * IW], I16, kind="ExternalInput")
    idx2_d = nc.dram_tensor("idx2", [P, NCH * IW], I16, kind="ExternalInput")
    swin_d = nc.dram_tensor("swin", [P, T * P], BF16, kind="ExternalInput")
    stw_d = nc.dram_tensor("stw", [P, T * P], BF16, kind="ExternalInput")
    out_d = nc.dram_tensor("out", [NPAD, OUT_C], F32, kind="ExternalOutput")

    hfull_d = nc.dram_tensor("hfull", [NROWS, F1R], BF16)
    h2loc_d = nc.dram_tensor("h2loc", [NPAD, PC], FP8)
    h2pkc_d = nc.dram_tensor("h2pkc", [NROWS, PC], FP8, addr_space="Shared")
    h2pk_d = nc.dram_tensor("h2pk", [NROWS, PR], FP8)

    rg = [list(range(NCORES))]

    with tile.TileContext(nc) as tc:
        with (
            tc.tile_pool(name="const", bufs=1) as cp,
            tc.tile_pool(name="persist", bufs=1) as pp,
        ):
            ident = cp.tile([P, P], BF16)
            make_identity(nc, ident[:])
            w1sb = cp.tile([P, KT1 * F1E], BF16)
            for k in range(KT1):
                nc.sync.dma_start(out=w1sb[:, k * F1E:(k + 1) * F1E], in_=w1e_d[k * P:(k + 1) * P, :])
            w2sb = cp.tile([P, KT2 * OCE], BF16)
            for k in range(KT2):
                nc.sync.dma_start(out=w2sb[:, k * OCE:(k + 1) * OCE], in_=w2e_d[k * P:(k + 1) * P, :])
            idx1sb = cp.tile([P, NCH * IW], I16)
            nc.gpsimd.dma_start(out=idx1sb[:], in_=idx1_d[:, :])
            idx2sb = cp.tile([P, NCH * IW], I16)
            zeros42 = cp.tile([P, W42], F32)
            nc.vector.memset(zeros42[:], 0.0)
            stwp_cm = tc.tile_pool(name="stwp", bufs=2)
            stwp = stwp_cm.__enter__()
            stwq_t = {}                        # rolling stw quarters (one per CHW windows)
            SQC = CHW * TPW * P                # stw cols per quarter
            QS = (T * P) // 4

            adw = pp.tile([P, W * H], BF16)    # a_d for local windows
            ad2w = pp.tile([P, W], BF16)       # layer-2 a_d for local windows
            adew2all = pp.tile([P, T], BF16)   # layer-2 a_d per edge (from MP1)
            h2all = pp.tile([P, W * PC], FP8)  # compact L2 gather rows (fp8)
            o2all = pp.tile([P, W * OUT_C], F32)
            mnall = pp.tile([P, W], F32)
            ssall = pp.tile([P, W], F32)

            # ---------------- Phase A: replicated GEMM1 -> hfull ----------------
            with (
                tc.tile_pool(name="slab", bufs=4) as slab,
                tc.tile_pool(name="psA", bufs=7, space="PSUM") as psA,
                tc.tile_pool(name="stg", bufs=4) as stg,
            ):
                for g in range(NSLAB):
                    xs = slab.tile([P, KT1 * SLAB * P], BF16, tag="xs")
                    for k in range(KT1):
                        nc.sync.dma_start(
                            out=xs[:, k * SLAB * P:(k + 1) * SLAB * P],
                            in_=xT_d[k * P:(k + 1) * P, g * SLAB * P:(g + 1) * SLAB * P],
                        )
                    for mm in range(SLAB):
                        m = g * SLAB + mm
                        ph = psA.tile([P, F1E], F32, tag="ph")
                        for k in range(KT1):
                            nc.tensor.matmul(
                                ph[:],
                                lhsT=xs[:, k * SLAB * P + mm * P: k * SLAB * P + (mm + 1) * P],
                                rhs=w1sb[:, k * F1E:(k + 1) * F1E],
                                start=(k == 0), stop=(k == KT1 - 1),
                            )
                        if mm % WB == 0:
                            hb = stg.tile([P, WB * F1A], BF16, tag="hb")
                        dst = hb[:, (mm % WB) * F1A:(mm % WB + 1) * F1A]
                        if mm % 2 == 0:
                            nc.scalar.activation(dst, ph[:, :F1A], AF.Copy)
                        else:
                            nc.vector.tensor_copy(out=dst, in_=ph[:, :F1A])
                        if m < W:
                            nc.vector.tensor_copy(out=adw[:, m * H:(m + 1) * H], in_=ph[:, F1 + H:F1E])
                        if mm % WB == WB - 1:
                            g8 = m // WB
                            nc.gpsimd.dma_start(
                                out=hfull_d[g8 * WB * P:(g8 + 1) * WB * P, :F1A]
                                    .rearrange("(t p) c -> p t c", p=P),
                                in_=hb[:].rearrange("p (t c) -> p t c", c=F1A),
                            )

            # idx2 load on SP after all xs loads (in-order queue
            # prevents the scheduler hoisting it into the startup window)
            nc.sync.dma_start(out=idx2sb[:], in_=idx2_d[:, :])

            # ---------------- Phase B: L1 message passing + GEMM2 ----------------
            with (
                tc.tile_pool(name="gp", bufs=8) as gp,
                tc.tile_pool(name="swp", bufs=3) as swp,
                tc.tile_pool(name="psAcc", bufs=3, space="PSUM") as psAcc,
                tc.tile_pool(name="psAde", bufs=3, space="PSUM") as psAde,
                tc.tile_pool(name="psT", bufs=1, space="PSUM") as psT,
                tc.tile_pool(name="psC", bufs=1, space="PSUM") as psC,
                tc.tile_pool(name="wp", bufs=2) as wp,
                tc.tile_pool(name="wps", bufs=3) as wps,
            ):
                chunk_tiles = {}

                def get_chunk(cc):
                    if cc in chunk_tiles:
                        return chunk_tiles[cc]
                    gt = gp.tile([P, GC * F1R], BF16, tag="gath")
                    nc.gpsimd.dma_gather(
                        out_ap=gt[:].rearrange("p (t f) -> p t f", f=F1R),
                        in_ap=hfull_d.ap(),
                        idxs_ap=idx1sb[:, cc * IW:(cc + 1) * IW],
                        num_idxs=NI, num_idxs_reg=NI, elem_size=F1R,
                        queue_num=cc % NSQ,
                    )
                    chunk_tiles[cc] = gt
                    return gt

                def segments(w):
                    """[(chunk, slot0, slot1, tl0)] covering tiles of window w."""
                    segs = []
                    t0, t1 = w * TPW, (w + 1) * TPW
                    t = t0
                    while t < t1:
                        cc = t // GC
                        s0 = t - cc * GC
                        s1 = min(GC, t1 - cc * GC)
                        segs.append((cc, s0, s1, t - t0))
                        t = cc * GC + s1
                    return segs

                for w in range(W):
                    if w % CHW == 0:        # stream stw quarter for these windows
                        q = w // CHW
                        stq = stwp.tile([P, SQC], BF16, tag="stwq")
                        nc.sync.dma_start(out=stq[:], in_=stw_d[:, q * SQC:(q + 1) * SQC])
                        stwq_t[q] = stq
                    if w == WA + 1:
                        # fire AllGather chunk A: its h2loc store landed
                        # ~2 windows ago, so the gpsimd queue does not
                        # stall on the wait
                        nc.gpsimd.collective_compute(
                            "AllGather", mybir.AluOpType.bypass, replica_groups=rg,
                            ins=[h2loc_d[:WA * P, :]],
                            outs=[h2pkc_d[:WA * P * NCORES, :]],
                        )
                    swin = swp.tile([P, TPW * P], BF16, tag="swin")
                    nc.sync.dma_start(
                        out=swin[:], in_=swin_d[:, w * TPW * P:(w + 1) * TPW * P])
                    segs = segments(w)
                    for (cc, s0, s1, tl0) in segs:
                        get_chunk(cc)
                    stq = stwq_t[w // CHW]
                    tb = (w % CHW) * TPW * P
                    adew = psAde.tile([P, TPW * H + TPW], F32, tag="adew")
                    for tl in range(TPW):
                        nc.tensor.matmul(
                            adew[:, tl * H:(tl + 1) * H],
                            lhsT=stq[:, tb + tl * P:tb + (tl + 1) * P],
                            rhs=adw[:, w * H:(w + 1) * H],
                            start=True, stop=True,
                        )
                    # scores: a_s read straight from the gather chunks;
                    # adew staged to SBUF bf16 (Act) so the adds run packed
                    adsb = wps.tile([P, TPW * H], BF16, tag="adsb")
                    nc.scalar.activation(adsb[:], adew[:, :TPW * H], AF.Copy)
                    escw = wps.tile([P, TPW * H], BF16, tag="escw")
                    for (cc, s0, s1, tl0) in segs:
                        gt = chunk_tiles[cc]
                        nc.vector.tensor_add(
                            out=escw[:, tl0 * H:(tl0 + s1 - s0) * H]
                                .rearrange("p (t h) -> p t h", h=H),
                            in0=adsb[:, tl0 * H:(tl0 + s1 - s0) * H]
                                .rearrange("p (t h) -> p t h", h=H),
                            in1=gt[:].rearrange("p (t f) -> p t f", f=F1R)[:, s0:s1, F1:F1A],
                        )
                    lrw = wps.tile([P, TPW * H], BF16, tag="lrw")
                    nc.scalar.activation(lrw[:], escw[:], AF.Prelu, alpha=NEG)
                    expw = wps.tile([P, TPW * H], BF16, tag="expw")
                    nc.scalar.activation(expw[:], lrw[:], AF.Exp)
                    # rhs = [h * alpha (head-interleaved) | alpha] per tile
                    rhs = wp.tile([P, TPW * F1A], BF16, tag="rhs")
                    for (cc, s0, s1, tl0) in segs:
                        gt = chunk_tiles[cc]
                        nt = s1 - s0
                        nc.vector.tensor_mul(
                            out=rhs[:].rearrange("p (t f) -> p t f", f=F1A)[:, tl0:tl0 + nt, :F1]
                                .rearrange("p t (c h) -> p t c h", h=H),
                            in0=gt[:].rearrange("p (t f) -> p t f", f=F1R)[:, s0:s1, :F1]
                                .rearrange("p t (c h) -> p t c h", h=H),
                            in1=expw[:].rearrange("p (t h) -> p t h", h=H)[:, tl0:tl0 + nt, :]
                                .rearrange("p t (o h) -> p t o h", o=1)
                                .to_broadcast([P, nt, C, H]),
                        )
                    nc.scalar.activation(
                        rhs[:].rearrange("p (t f) -> p t f", f=F1A)[:, :, F1:F1A],
                        expw[:].rearrange("p (t h) -> p t h", h=H), AF.Copy)
                    acc = psAcc.tile([P, F1A], F32, tag="acc")
                    for tl in range(TPW):
                        nc.tensor.matmul(
                            acc[:], lhsT=swin[:, tl * P:(tl + 1) * P],
                            rhs=rhs[:, tl * F1A:(tl + 1) * F1A],
                            start=(tl == 0), stop=(tl == TPW - 1),
                        )
                    # finalize: o1 = acc/z, ELU, transpose, GEMM2
                    zs = wps.tile([P, H], F32, tag="zs")
                    nc.vector.tensor_scalar_add(out=zs[:], in0=acc[:, F1:F1A], scalar1=1e-16)
                    zr = wps.tile([P, H], F32, tag="zr")
                    nc.vector.reciprocal(zr[:], zs[:])
                    o1 = wps.tile([P, F1], F32, tag="o1")
                    nc.vector.tensor_mul(
                        out=o1[:].rearrange("p (c h) -> p c h", h=H),
                        in0=acc[:, :F1].rearrange("p (c h) -> p c h", h=H),
                        in1=zr[:].rearrange("p (o h) -> p o h", o=1).to_broadcast([P, C, H]),
                    )
                    rneg = wps.tile([P, F1], F32, tag="rneg")
                    nc.scalar.activation(rneg[:], o1[:], AF.Relu, scale=-1.0)
                    em = wps.tile([P, F1], F32, tag="em")
                    nc.scalar.activation(em[:], rneg[:], AF.Exp, scale=-1.0)
                    mx = wps.tile([P, F1], F32, tag="mx")
                    nc.scalar.activation(mx[:], o1[:], AF.Relu)
                    h1 = wps.tile([P, F1], BF16, tag="h1")
                    nc.vector.scalar_tensor_tensor(
                        out=h1[:], in0=em[:], scalar=-1.0, in1=mx[:],
                        op0=mybir.AluOpType.add, op1=mybir.AluOpType.add,
                    )
                    h1Tw = wps.tile([P, KT2 * P], BF16, tag="h1Tw")
                    for fc in range(KT2):
                        tp = psT.tile([P, P], BF16, tag="tp")
                        nc.tensor.transpose(tp[:], h1[:, fc * P:(fc + 1) * P], ident[:])
                        nc.scalar.activation(h1Tw[:, fc * P:(fc + 1) * P], tp[:], AF.Copy)
                    p2 = psC.tile([P, OCE], F32, tag="p2")
                    for k in range(KT2):
                        nc.tensor.matmul(
                            p2[:],
                            lhsT=h1Tw[:, k * P:(k + 1) * P],
                            rhs=w2sb[:, k * OCE:(k + 1) * OCE],
                            start=(k == 0), stop=(k == KT2 - 1),
                        )
                    # build the padded fp8 L2 gather row for this window:
                    # [h2(40) | 1.0 | pad | a_s2 x42 | pad]
                    nc.scalar.activation(
                        h2all[:, w * PC:w * PC + OUT_C], p2[:, :OUT_C], AF.Copy)
                    nc.vector.memset(h2all[:, w * PC + OUT_C:w * PC + OUT_C + 1], 1.0)
                    as2sb = wps.tile([P, 1], F32, tag="as2sb")
                    nc.vector.tensor_copy(out=as2sb[:], in_=p2[:, OUT_C:OUT_C + 1])
                    nc.scalar.activation(
                        h2all[:, w * PC + AS0:w * PC + AS1], zeros42[:],
                        AF.Identity, bias=as2sb[:])
                    nc.vector.tensor_copy(out=ad2w[:, w:w + 1], in_=p2[:, OCE - 1:OCE])
                    # layer-2 a_d per edge, computed now while stw tile is hot
                    for tl in range(TPW):
                        nc.tensor.matmul(
                            adew[:, TPW * H + tl:TPW * H + tl + 1],
                            lhsT=stq[:, tb + tl * P:tb + (tl + 1) * P],
                            rhs=ad2w[:, w:w + 1],
                            start=True, stop=True,
                        )
                    nc.scalar.activation(
                        adew2all[:, w * TPW:(w + 1) * TPW], adew[:, TPW * H:], AF.Copy)
                    # ship AllGather chunk inputs while MP1 continues:
                    # big chunk A = windows [0, WA), small tail chunk B
                    if w == WA - 1:
                        nc.sync.dma_start(
                            out=h2loc_d[:WA * P, :]
                                .rearrange("(t p) c -> p t c", p=P),
                            in_=h2all[:, :WA * PC]
                                .rearrange("p (t c) -> p t c", c=PC),
                        )
                    if w == W - 1:
                        nc.sync.dma_start(
                            out=h2loc_d[WA * P:, :]
                                .rearrange("(t p) c -> p t c", p=P),
                            in_=h2all[:, WA * PC:]
                                .rearrange("p (t c) -> p t c", c=PC),
                        )
                # tail AllGather chunk fires right after its store
                nc.gpsimd.collective_compute(
                    "AllGather", mybir.AluOpType.bypass, replica_groups=rg,
                    ins=[h2loc_d[WA * P:, :]],
                    outs=[h2pkc_d[WA * P * NCORES:, :]],
                )

            stwp_cm.__exit__(None, None, None)   # stw quarters dead
            swall_cm = tc.tile_pool(name="swall", bufs=1)
            swallp = swall_cm.__enter__()
            swall = swallp.tile([P, T * P], BF16)
            for q in range(4):
                nc.sync.dma_start(out=swall[:, q * QS:(q + 1) * QS],
                                  in_=swin_d[:, q * QS:(q + 1) * QS])
            # expand compact 96B AllGather rows to 256B-padded gather rows
            # (sub-chunked so chunk-A expands overlap the tail AllGather)
            XR = 2 * P * NCORES // P  # rows per partition per expand (2 windows)
            with tc.tile_pool(name="xp", bufs=2) as xp:
                for x0 in range(0, W, 2):
                    r0, r1 = x0 * P * NCORES, (x0 + 2) * P * NCORES
                    e96 = xp.tile([P, XR * PC], FP8, tag="e96")
                    nc.gpsimd.dma_start(
                        out=e96[:].rearrange("p (t c) -> p t c", c=PC),
                        in_=h2pkc_d[r0:r1, :].rearrange("(p t) c -> p t c", p=P))
                    e256 = xp.tile([P, XR * PR], FP8, tag="e256")
                    nc.vector.tensor_copy(
                        out=e256[:].rearrange("p (t c) -> p t c", c=PR)[:, :, :PC],
                        in_=e96[:].rearrange("p (t c) -> p t c", c=PC))
                    nc.sync.dma_start(
                        out=h2pk_d[r0:r1, :].rearrange("(p t) c -> p t c", p=P),
                        in_=e256[:].rearrange("p (t c) -> p t c", c=PR))

            # ---------------- Phase D: layer-2 message passing ----------------
            with (
                tc.tile_pool(name="gp2", bufs=10) as gp2,
                tc.tile_pool(name="gbp", bufs=10) as gbp,
                tc.tile_pool(name="psAcc2", bufs=2, space="PSUM") as psAcc2,
                tc.tile_pool(name="wp2", bufs=2) as wp2,
                tc.tile_pool(name="wps2", bufs=3) as wps2,
                tc.tile_pool(name="outp", bufs=2) as outp,
            ):
                chunk2 = {}

                def get_chunk2(cc):
                    if cc in chunk2:
                        return chunk2[cc]
                    g2 = gp2.tile([P, GC * PR], FP8, tag="gath2")
                    nc.gpsimd.dma_gather(
                        out_ap=g2[:].rearrange("p (t f) -> p t f", f=PR),
                        in_ap=h2pk_d.ap(),
                        idxs_ap=idx2sb[:, cc * IW:(cc + 1) * IW],
                        num_idxs=NI, num_idxs_reg=NI, elem_size=PR,
                        queue_num=cc % NSQ,
                    )
                    gb = gbp.tile([P, GC * CW], BF16, tag="gbf")
                    nc.scalar.activation(
                        gb[:].rearrange("p (t f) -> p t f", f=CW),
                        g2[:].rearrange("p (t f) -> p t f", f=PR)[:, :, :CW], AF.Copy)
                    chunk2[cc] = gb
                    return gb

                def segments2(w):
                    segs = []
                    t0, t1 = w * TPW, (w + 1) * TPW
                    t = t0
                    while t < t1:
                        cc = t // GC
                        s0 = t - cc * GC
                        s1 = min(GC, t1 - cc * GC)
                        segs.append((cc, s0, s1, t - t0))
                        t = cc * GC + s1
                    return segs

                for w in range(W):
                    swin2 = swall[:, w * TPW * P:(w + 1) * TPW * P]
                    segs = segments2(w)
                    for (cc, s0, s1, tl0) in segs:
                        get_chunk2(cc)
                    # packed 42-wide: esc = a_s2(rep) + a_d2(bcast); then
                    # Prelu/Exp give alpha replicated x42 with no picks
                    esc2w = wps2.tile([P, TPW * W42], BF16, tag="esc2w")
                    for (cc, s0, s1, tl0) in segs:
                        gb = chunk2[cc]
                        nt = s1 - s0
                        nc.vector.tensor_add(
                            out=esc2w[:].rearrange("p (t f) -> p t f", f=W42)[:, tl0:tl0 + nt, :],
                            in0=gb[:].rearrange("p (t f) -> p t f", f=CW)[:, s0:s1, AS0:AS1],
                            in1=adew2all[:, w * TPW + tl0:w * TPW + tl0 + nt]
                                .rearrange("p (t o) -> p t o", o=1)
                                .to_broadcast([P, nt, W42]),
                        )
                    lr2w = wps2.tile([P, TPW * W42], BF16, tag="lr2w")
                    nc.scalar.activation(lr2w[:], esc2w[:], AF.Prelu, alpha=NEG)
                    exp2w = wps2.tile([P, TPW * W42], BF16, tag="exp2w")
                    nc.scalar.activation(exp2w[:], lr2w[:], AF.Exp)
                    # rhs = [h2 | 1] * alpha, all packed 42-wide
                    rhs2 = wp2.tile([P, TPW * W42], BF16, tag="rhs2")
                    for (cc, s0, s1, tl0) in segs:
                        gb = chunk2[cc]
                        nt = s1 - s0
                        nc.vector.tensor_mul(
                            out=rhs2[:].rearrange("p (t f) -> p t f", f=W42)[:, tl0:tl0 + nt, :],
                            in0=gb[:].rearrange("p (t f) -> p t f", f=CW)[:, s0:s1, :W42],
                            in1=exp2w[:].rearrange("p (t f) -> p t f", f=W42)[:, tl0:tl0 + nt, :],
                        )
                    acc2 = psAcc2.tile([P, OC1], F32, tag="acc2")
                    for tl in range(TPW):
                        nc.tensor.matmul(
                            acc2[:], lhsT=swin2[:, tl * P:(tl + 1) * P],
                            rhs=rhs2[:, tl * W42:tl * W42 + OC1],
                            start=(tl == 0), stop=(tl == TPW - 1),
                        )
                    zs2 = wps2.tile([P, 1], F32, tag="zs2")
                    nc.vector.tensor_scalar_add(out=zs2[:], in0=acc2[:, OUT_C:OC1], scalar1=1e-16)
                    zr2 = wps2.tile([P, 1], F32, tag="zr2")
                    nc.vector.reciprocal(zr2[:], zs2[:])
                    nc.vector.tensor_mul(
                        out=o2all[:, w * OUT_C:(w + 1) * OUT_C], in0=acc2[:, :OUT_C],
                        in1=zr2[:].to_broadcast([P, OUT_C]),
                    )
                    nc.vector.tensor_reduce(
                        out=mnall[:, w:w + 1], in_=o2all[:, w * OUT_C:(w + 1) * OUT_C],
                        axis=mybir.AxisListType.X,
                        op=mybir.AluOpType.max, negate=True,
                    )
                    ex = wps2.tile([P, OUT_C], F32, tag="ex")
                    nc.scalar.activation(
                        ex[:], o2all[:, w * OUT_C:(w + 1) * OUT_C], AF.Exp,
                        bias=mnall[:, w:w + 1], accum_out=ssall[:, w:w + 1],
                    )

                # single Ln pass over all windows, then per-window bias add
                lnall = wps2.tile([P, W], F32, tag="lnall")
                nc.scalar.activation(lnall[:], ssall[:], AF.Ln)
                comb = wps2.tile([P, W], F32, tag="comb")
                nc.vector.tensor_sub(out=comb[:], in0=mnall[:], in1=lnall[:])
                for w in range(W):
                    if w % 4 == 0:
                        fin4 = outp.tile([P, 4 * OUT_C], F32, tag="fin4")
                    nc.scalar.activation(
                        fin4[:, (w % 4) * OUT_C:(w % 4 + 1) * OUT_C],
                        o2all[:, w * OUT_C:(w + 1) * OUT_C],
                        AF.Identity, bias=comb[:, w:w + 1])
                    if w % 4 == 3:
                        g4 = w // 4
                        nc.scalar.dma_start(
                            out=out_d[g4 * 4 * P:(g4 + 1) * 4 * P, :]
                                .rearrange("(t p) c -> p t c", p=P),
                            in_=fin4[:].rearrange("p (t c) -> p t c", c=OUT_C),
                        )
            swall_cm.__exit__(None, None, None)

    nc.compile()
    return nc


def _prepare(x, edge_index, w1, att_src1, att_dst1, b1, w2, att_src2, att_dst2, b2):
    x = np.asarray(x, dtype=np.float32)
    edge_index = np.asarray(edge_index)
    w1 = np.asarray(w1, dtype=np.float32)
    att_src1 = np.asarray(att_src1, dtype=np.float32)
    att_dst1 = np.asarray(att_dst1, dtype=np.float32)
    b1 = np.asarray(b1, dtype=np.float32)
    w2 = np.asarray(w2, dtype=np.float32)
    att_src2 = np.asarray(att_src2, dtype=np.float32)
    att_dst2 = np.asarray(att_dst2, dtype=np.float32)
    b2 = np.asarray(b2, dtype=np.float32)
    assert not np.any(b1) and not np.any(b2), "nonzero bias unsupported"

    N, IN_C = x.shape
    H, C = att_src1.shape
    F1 = H * C
    OUT_C = w2.shape[1]
    assert N % NCORES == 0
    NPC = N // NCORES
    W = (NPC + P - 1) // P
    NPAD = W * P
    NROWS = NCORES * NPAD
    assert NROWS < 32768
    assert W % CHW == 0

    # ---- edges: append self-loops, sort by destination ----
    src = np.concatenate([edge_index[0].astype(np.int64), np.arange(N, dtype=np.int64)])
    dst = np.concatenate([edge_index[1].astype(np.int64), np.arange(N, dtype=np.int64)])
    order = np.argsort(dst, kind="stable")
    src, dst = src[order], dst[order]

    core_of = dst // NPC
    bounds = np.searchsorted(dst, np.arange(NCORES + 1) * NPC)
    win_of = (dst - core_of * NPC) // P

    counts = np.zeros((NCORES, W), np.int64)
    for cidx in range(NCORES):
        w_arr = win_of[bounds[cidx]:bounds[cidx + 1]]
        counts[cidx] = np.bincount(w_arr, minlength=W)
    TPW = max(1, int(np.ceil(counts.max() / P)))
    T = W * TPW
    TG = ((T + GC - 1) // GC) * GC

    blocked = (src // NPC) * NPAD + (src % NPC)     # global padded row of src

    # layer-2 rows live in AllGather-chunked layout: chunk A = padded local
    # rows [0, CA) of every core, concatenated by rank; chunk B = the rest
    WA = W - 6                 # must match _build_program
    CA = WA * P
    src_c = src // NPC
    src_r = src % NPC
    in_a = src_r < CA
    blocked2 = np.where(
        in_a,
        src_c * CA + src_r,
        CA * NCORES + src_c * (NPAD - CA) + (src_r - CA),
    )

    NI = GC * P
    IW = NI // 16
    NCH = TG // GC

    def pack_idx(ids):
        """gather-index layout: chunk cc's idxs at cols [cc*IW,(cc+1)*IW)."""
        idx16 = np.zeros((16, NCH * IW), np.int16)
        gpos = np.arange(TG * P)
        cc, ii = gpos // NI, gpos % NI
        full = np.zeros(TG * P, np.int16)
        full[:len(ids)] = ids
        idx16[ii % 16, cc * IW + ii // 16] = full
        return np.tile(idx16, (8, 1))

    in_maps = []
    xTf = np.zeros((IN_C, NROWS), np.float32)
    xTf = xTf.reshape(IN_C, NCORES, NPAD)
    xTf[:, :, :NPC] = x.T.reshape(IN_C, NCORES, NPC)
    xTf = xTf.reshape(IN_C, NROWS)
    xTf_bf = _to_bf(xTf)

    # head-interleaved permutation: new col c*H+h <- old col h*C+c
    f_old = np.arange(F1)
    h_idx, c_idx = f_old // C, f_old % C
    f_new = c_idx * H + h_idx
    perm = np.empty(F1, np.int64)
    perm[f_new] = f_old          # perm[new] = old

    Asrc = np.zeros((F1, H), np.float32)
    Adst = np.zeros((F1, H), np.float32)
    for h in range(H):
        Asrc[h * C:(h + 1) * C, h] = att_src1[h]
        Adst[h * C:(h + 1) * C, h] = att_dst1[h]
    w1P = w1[:, perm]
    w1e = np.concatenate([w1P, w1 @ Asrc, w1 @ Adst], axis=1)
    w2P = w2[perm, :]
    w2e = np.concatenate([w2P, w2P @ att_src2.T, w2P @ att_dst2.T], axis=1)
    w1e_bf = _to_bf(w1e)
    w2e_bf = _to_bf(w2e)

    nc = _build_program(IN_C, F1, H, C, OUT_C, NPAD, T, TPW, W, TG, NROWS)

    for cidx in range(NCORES):
        ids_g = np.zeros(T * P, np.int64)         # L1: global padded row per slot
        ids_g2 = np.zeros(T * P, np.int64)        # L2: chunked-AllGather row per slot
        dloc = np.full(T * P, 255, np.int64)      # pad -> no one-hot match
        s_c = blocked[bounds[cidx]:bounds[cidx + 1]]
        s2_c = blocked2[bounds[cidx]:bounds[cidx + 1]]
        w_c = win_of[bounds[cidx]:bounds[cidx + 1]]
        d_c = dst[bounds[cidx]:bounds[cidx + 1]] - cidx * NPC
        wb = np.searchsorted(w_c, np.arange(W + 1))
        for w in range(W):
            n = wb[w + 1] - wb[w]
            base = w * TPW * P
            ids_g[base:base + n] = s_c[wb[w]:wb[w + 1]]
            ids_g2[base:base + n] = s2_c[wb[w]:wb[w + 1]]
            dloc[base:base + n] = d_c[wb[w]:wb[w + 1]] - w * P
        ids_rot = (ids_g - cidx * NPAD) % NROWS   # L1 rows are core-rotated

        # one-hot scatter/gather matrices
        M = np.zeros((T * P, P), np.float32)
        real = dloc < P
        M[np.nonzero(real)[0], dloc[real]] = 1.0
        M3 = M.reshape(T, P, P)
        swin = _to_bf(np.ascontiguousarray(M3.transpose(1, 0, 2)).reshape(P, T * P))
        stw_m = _to_bf(np.ascontiguousarray(M3.transpose(2, 0, 1)).reshape(P, T * P))

        in_maps.append({
            "xT": np.ascontiguousarray(np.roll(xTf_bf, -cidx * NPAD, axis=1)) if cidx else xTf_bf,
            "w1e": w1e_bf,
            "w2e": w2e_bf,
            "idx1": pack_idx(ids_rot.astype(np.int16)),
            "idx2": pack_idx(ids_g2.astype(np.int16)),
            "swin": swin,
            "stw": stw_m,
        })
    return nc, in_maps, NPC


def kernel(_trace=False, **inputs):
    nc, in_maps, NPC = _prepare(**inputs)
    res = run_bass_kernel_spmd(nc, in_maps, core_ids=list(range(NCORES)), trace=_trace)
    out = np.concatenate([res.results[cidx]["out"][:NPC] for cidx in range(NCORES)], axis=0)
    kernel.last_exec_time_ns = res.exec_time_ns
    kernel.last_res = res
    return out.astype(np.float32)


# revision 8
# speedup vs baseline: 1.1413x; 1.1206x over previous
"""Two-layer GAT on 8 Trainium2 NeuronCores.

Strategy: collective-minimal, SPMD over destination ranges.
- Layer-1 GEMM (x@w1) is REPLICATED: every core computes h for all N nodes
  and writes gather-ready packed rows [h | a_s] straight into its own DRAM.
  A per-core rotation of the node-row space keeps the SPMD program identical
  while placing each core's own destination windows at rows 0..NPAD-1.
- Edges (with self-loops) are sorted by destination; core c owns dst range
  [c*NPC, (c+1)*NPC) and computes those output rows entirely locally.
- Gathers use 1024-index chunks round-robined over 4 SWDGE queues: a single
  qPoolDynamic ring drains ~8.6us per 1024 descriptors regardless of element
  size, and the rings drain independently, so 4 queues give ~2.2us/chunk.
  (GpSimd desc-gen itself is only ~1.4us/chunk and is not the serializer.)
- Layer-2 rows are built DURING MP1 as 256B-padded fp8 gather rows held in
  SBUF: [h2(40) | 1.0 | pad | a_s2 replicated x42 | pad]. The 1.0 column
  makes the softmax normalizer fall out of the scatter matmul for free; the
  a_s2 replication (Act per-partition-bias broadcast) turns MP2's score adds
  and alpha scaling into fully PACKED 42-wide DVE ops -- the old per-edge
  strided column picks cost ~700ns/element on DVE slow mode (~210us total).
- The layer-2 AllGather ships the padded fp8 rows in 4 chunks of 5 windows,
  each fired as soon as its windows finish, overlapping MP1; MP2 gathers
  fp8 directly (transpose=False gathers are byte movers; the old bf16
  expand chain cost ~140us of tiny strided DMA descriptors).
- One-hot scatter (Swin) / gather (STw) matrices are precomputed on the
  HOST and shipped as bf16: STw stays resident in SBUF for MP1 and also
  distributes layer-2's a_d per edge (adew2all), so MP2 needs no STw.
- Features are head-interleaved (f = c*H + h) so per-edge message scaling
  broadcasts alpha along a packed 8-wide run (fast DVE mode).
- Act engine: Prelu/Exp/Relu from one table; the final log(sum) runs ONCE
  over all windows after the loop.
"""
import sys

sys.path.insert(0, "/opt/trn_rl_repo")
import numpy as np
import ml_dtypes

import concourse.bass as bass
import concourse.bacc as bacc
import concourse.mybir as mybir
import concourse.tile as tile
from concourse.bass_utils import run_bass_kernel_spmd
from concourse.masks import make_identity

BF16 = mybir.dt.bfloat16
F32 = mybir.dt.float32
I16 = mybir.dt.int16
FP8 = mybir.dt.float8e4
nbf16 = ml_dtypes.bfloat16
AF = mybir.ActivationFunctionType

NCORES = 8
NEG = 0.2
P = 128
GC = 8          # edge tiles per gather chunk (1024 idxs = SWDGE ring cap)
NSQ = 4         # SWDGE queues (hardware max)
CHW = 5         # windows per AllGather chunk
PR = 256        # padded fp8 gather row bytes for layer 2
PC = 96         # compact fp8 row width shipped through the AllGather
CW = 96         # fp8->bf16 cast width (covers h2|1|pad|a_s2 rep)
AS0, AS1 = 48, 90   # a_s2 replicated at fp8 cols [48, 90)


def _to_bf(a):
    return np.ascontiguousarray(np.asarray(a, dtype=np.float32).astype(nbf16))


def _build_program(IN_C, F1, H, C, OUT_C, NPAD, T, TPW, W, TG, NROWS):
    KT1 = IN_C // P            # 4 k-tiles for GEMM1
    KT2 = F1 // P              # 2 k-tiles for GEMM2
    F1E = F1 + 2 * H           # gemm1 columns: [w1 | w1@As | w1@Ad]
    F1A = F1 + H               # packed row content: [h | a_s]
    F1R = ((F1A * 2 + 255) // 256) * 128   # L1 gather row width (bf16)
    OCE = OUT_C + 2            # gemm2 columns: [w2 | w2@as2 | w2@ad2]
    OC1 = OUT_C + 1            # layer-2 scatter rhs width consumed: [msg | exp]
    W42 = OUT_C + 2            # packed 42-wide lane for L2 score/alpha ops
    MT = NROWS // P            # 160 m-tiles in replicated GEMM1
    SLAB = 8                   # m-tiles per xT slab
    NSLAB = MT // SLAB
    WB = 4                     # m-tiles per hfull write
    NCH = TG // GC             # gather chunks
    NI = GC * P                # idxs per gather
    IW = NI // 16              # idx cols per chunk
    NQC = W // CHW             # AllGather chunks
    CR = CHW * P               # rows per AllGather chunk (per core)
    WA = W - 6                 # windows in the early (hidden) AllGather chunk

    nc = bacc.Bacc(num_devices=NCORES, num_swdge_queues=NSQ)

    xT_d = nc.dram_tensor("xT", [IN_C, NROWS], BF16, kind="ExternalInput")
    w1e_d = nc.dram_tensor("w1e", [IN_C, F1E], BF16, kind="ExternalInput")
    w2e_d = nc.dram_tensor("w2e", [F1, OCE], BF16, kind="ExternalInput")
    idx1_d = nc.dram_tensor("idx1", [P, NCH * IW], I16, kind="ExternalInput")
    idx2_d = nc.dram_tensor("idx2", [P, NCH * IW], I16, kind="ExternalInput")
    swin_d = nc.dram_tensor("swin", [P, T * P], BF16, kind="ExternalInput")
    stw_d = nc.dram_tensor("stw", [P, T * P], BF16, kind="ExternalInput")
    out_d = nc.dram_tensor("out", [NPAD, OUT_C], F32, kind="ExternalOutput")

    hfull_d = nc.dram_tensor("hfull", [NROWS, F1R], BF16)
    h2loc_d = nc.dram_tensor("h2loc", [NPAD, PC], FP8)
    h2pkc_d = nc.dram_tensor("h2pkc", [NROWS, PC], FP8, addr_space="Shared")
    h2pk_d = nc.dram_tensor("h2pk", [NROWS, PR], FP8)

    rg = [list(range(NCORES))]

    with tile.TileContext(nc) as tc:
        with (
            tc.tile_pool(name="const", bufs=1) as cp,
            tc.tile_pool(name="persist", bufs=1) as pp,
        ):
            ident = cp.tile([P, P], BF16)
            make_identity(nc, ident[:])
            w1sb = cp.tile([P, KT1 * F1E], BF16)
            for k in range(KT1):
                nc.sync.dma_start(out=w1sb[:, k * F1E:(k + 1) * F1E], in_=w1e_d[k * P:(k + 1) * P, :])
            w2sb = cp.tile([P, KT2 * OCE], BF16)
            for k in range(KT2):
                nc.sync.dma_start(out=w2sb[:, k * OCE:(k + 1) * OCE], in_=w2e_d[k * P:(k + 1) * P, :])
            idx1sb = cp.tile([P, NCH * IW], I16)
            nc.gpsimd.dma_start(out=idx1sb[:], in_=idx1_d[:, :])
            idx2sb = cp.tile([P, NCH * IW], I16)
            zeros42 = cp.tile([P, W42], F32)
            nc.vector.memset(zeros42[:], 0.0)
            stwp_cm = tc.tile_pool(name="stwp", bufs=2)
            stwp = stwp_cm.__enter__()
            stwq_t = {}                        # rolling stw quarters (one per CHW windows)
            SQC = CHW * TPW * P                # stw cols per quarter
            QS = (T * P) // 4

            adw = pp.tile([P, W * H], BF16)    # a_d for local windows
            ad2w = pp.tile([P, W], BF16)       # layer-2 a_d for local windows
            adew2all = pp.tile([P, T], BF16)   # layer-2 a_d per edge (from MP1)
            h2all = pp.tile([P, W * PC], FP8)  # compact L2 gather rows (fp8)
            o2all = pp.tile([P, W * OUT_C], F32)
            mnall = pp.tile([P, W], F32)
            ssall = pp.tile([P, W], F32)

            # ---------------- Phase A: replicated GEMM1 -> hfull ----------------
            with (
                tc.tile_pool(name="slab", bufs=4) as slab,
                tc.tile_pool(name="psA", bufs=7, space="PSUM") as psA,
                tc.tile_pool(name="stg", bufs=4) as stg,
            ):
                for g in range(NSLAB):
                    xs = slab.tile([P, KT1 * SLAB * P], BF16, tag="xs")
                    for k in range(KT1):
                        nc.sync.dma_start(
                            out=xs[:, k * SLAB * P:(k + 1) * SLAB * P],
                            in_=xT_d[k * P:(k + 1) * P, g * SLAB * P:(g + 1) * SLAB * P],
                        )
                    for mm in range(SLAB):
                        m = g * SLAB + mm
                        ph = psA.tile([P, F1E], F32, tag="ph")
                        for k in range(KT1):
                            nc.tensor.matmul(
                                ph[:],
                                lhsT=xs[:, k * SLAB * P + mm * P: k * SLAB * P + (mm + 1) * P],
                                rhs=w1sb[:, k * F1E:(k + 1) * F1E],
                                start=(k == 0), stop=(k == KT1 - 1),
                            )
                        if mm % WB == 0:
                            hb = stg.tile([P, WB * F1A], BF16, tag="hb")
                        dst = hb[:, (mm % WB) * F1A:(mm % WB + 1) * F1A]
                        if mm % 2 == 0:
                            nc.scalar.activation(dst, ph[:, :F1A], AF.Copy)
                        else:
                            nc.vector.tensor_copy(out=dst, in_=ph[:, :F1A])
                        if m < W:
                            nc.vector.tensor_copy(out=adw[:, m * H:(m + 1) * H], in_=ph[:, F1 + H:F1E])
                        if mm % WB == WB - 1:
                            g8 = m // WB
                            nc.gpsimd.dma_start(
                                out=hfull_d[g8 * WB * P:(g8 + 1) * WB * P, :F1A]
                                    .rearrange("(t p) c -> p t c", p=P),
                                in_=hb[:].rearrange("p (t c) -> p t c", c=F1A),
                            )

            # idx2 load on SP after all xs loads (in-order queue
            # prevents the scheduler hoisting it into the startup window)
            nc.sync.dma_start(out=idx2sb[:], in_=idx2_d[:, :])

            # ---------------- Phase B: L1 message passing + GEMM2 ----------------
            with (
                tc.tile_pool(name="gp", bufs=8) as gp,
                tc.tile_pool(name="swp", bufs=3) as swp,
                tc.tile_pool(name="psAcc", bufs=3, space="PSUM") as psAcc,
                tc.tile_pool(name="psAde", bufs=3, space="PSUM") as psAde,
                tc.tile_pool(name="psT", bufs=1, space="PSUM") as psT,
                tc.tile_pool(name="psC", bufs=1, space="PSUM") as psC,
                tc.tile_pool(name="wp", bufs=2) as wp,
                tc.tile_pool(name="wps", bufs=3) as wps,
            ):
                chunk_tiles = {}

                def get_chunk(cc):
                    if cc in chunk_tiles:
                        return chunk_tiles[cc]
                    gt = gp.tile([P, GC * F1R], BF16, tag="gath")
                    nc.gpsimd.dma_gather(
                        out_ap=gt[:].rearrange("p (t f) -> p t f", f=F1R),
                        in_ap=hfull_d.ap(),
                        idxs_ap=idx1sb[:, cc * IW:(cc + 1) * IW],
                        num_idxs=NI, num_idxs_reg=NI, elem_size=F1R,
                        queue_num=cc % NSQ,
                    )
                    chunk_tiles[cc] = gt
                    return gt

                def segments(w):
                    """[(chunk, slot0, slot1, tl0)] covering tiles of window w."""
                    segs = []
                    t0, t1 = w * TPW, (w + 1) * TPW
                    t = t0
                    while t < t1:
                        cc = t // GC
                        s0 = t - cc * GC
                        s1 = min(GC, t1 - cc * GC)
                        segs.append((cc, s0, s1, t - t0))
                        t = cc * GC + s1
                    return segs

                for w in range(W):
                    if w % CHW == 0:        # stream stw quarter for these windows
                        q = w // CHW
                        stq = stwp.tile([P, SQC], BF16, tag="stwq")
                        nc.sync.dma_start(out=stq[:], in_=stw_d[:, q * SQC:(q + 1) * SQC])
                        stwq_t[q] = stq
                    if w >= 6 and (w - 6) % CHW == 0:
                        # fire AllGather chunk (w-6)//CHW: its h2loc store
                        # landed ~1.5 windows ago, so the gpsimd queue
                        # does not stall on the wait
                        q = (w - 6) // CHW
                        nc.gpsimd.collective_compute(
                            "AllGather", mybir.AluOpType.bypass, replica_groups=rg,
                            ins=[h2loc_d[q * CR:(q + 1) * CR, :]],
                            outs=[h2pkc_d[q * CR * NCORES:(q + 1) * CR * NCORES, :]],
                        )
                    swin = swp.tile([P, TPW * P], BF16, tag="swin")
                    nc.sync.dma_start(
                        out=swin[:], in_=swin_d[:, w * TPW * P:(w + 1) * TPW * P])
                    segs = segments(w)
                    for (cc, s0, s1, tl0) in segs:
                        get_chunk(cc)
                    stq = stwq_t[w // CHW]
                    tb = (w % CHW) * TPW * P
                    adew = psAde.tile([P, TPW * H + TPW], F32, tag="adew")
                    for tl in range(TPW):
                        nc.tensor.matmul(
                            adew[:, tl * H:(tl + 1) * H],
                            lhsT=stq[:, tb + tl * P:tb + (tl + 1) * P],
                            rhs=adw[:, w * H:(w + 1) * H],
                            start=True, stop=True,
                        )
                    # scores: a_s read straight from the gather chunks;
                    # adew staged to SBUF bf16 (Act) so the adds run packed
                    adsb = wps.tile([P, TPW * H], BF16, tag="adsb")
                    nc.scalar.activation(adsb[:], adew[:, :TPW * H], AF.Copy)
                    escw = wps.tile([P, TPW * H], BF16, tag="escw")
                    for (cc, s0, s1, tl0) in segs:
                        gt = chunk_tiles[cc]
                        nc.vector.tensor_add(
                            out=escw[:, tl0 * H:(tl0 + s1 - s0) * H]
                                .rearrange("p (t h) -> p t h", h=H),
                            in0=adsb[:, tl0 * H:(tl0 + s1 - s0) * H]
                                .rearrange("p (t h) -> p t h", h=H),
                            in1=gt[:].rearrange("p (t f) -> p t f", f=F1R)[:, s0:s1, F1:F1A],
                        )
                    lrw = wps.tile([P, TPW * H], BF16, tag="lrw")
                    nc.scalar.activation(lrw[:], escw[:], AF.Prelu, alpha=NEG)
                    expw = wps.tile([P, TPW * H], BF16, tag="expw")
                    nc.scalar.activation(expw[:], lrw[:], AF.Exp)
                    # rhs = [h * alpha (head-interleaved) | alpha] per tile
                    rhs = wp.tile([P, TPW * F1A], BF16, tag="rhs")
                    for (cc, s0, s1, tl0) in segs:
                        gt = chunk_tiles[cc]
                        nt = s1 - s0
                        nc.vector.tensor_mul(
                            out=rhs[:].rearrange("p (t f) -> p t f", f=F1A)[:, tl0:tl0 + nt, :F1]
                                .rearrange("p t (c h) -> p t c h", h=H),
                            in0=gt[:].rearrange("p (t f) -> p t f", f=F1R)[:, s0:s1, :F1]
                                .rearrange("p t (c h) -> p t c h", h=H),
                            in1=expw[:].rearrange("p (t h) -> p t h", h=H)[:, tl0:tl0 + nt, :]
                                .rearrange("p t (o h) -> p t o h", o=1)
                                .to_broadcast([P, nt, C, H]),
                        )
                    nc.scalar.activation(
                        rhs[:].rearrange("p (t f) -> p t f", f=F1A)[:, :, F1:F1A],
                        expw[:].rearrange("p (t h) -> p t h", h=H), AF.Copy)
                    acc = psAcc.tile([P, F1A], F32, tag="acc")
                    for tl in range(TPW):
                        nc.tensor.matmul(
                            acc[:], lhsT=swin[:, tl * P:(tl + 1) * P],
                            rhs=rhs[:, tl * F1A:(tl + 1) * F1A],
                            start=(tl == 0), stop=(tl == TPW - 1),
                        )
                    # finalize: o1 = acc/z, ELU, transpose, GEMM2
                    zs = wps.tile([P, H], F32, tag="zs")
                    nc.vector.tensor_scalar_add(out=zs[:], in0=acc[:, F1:F1A], scalar1=1e-16)
                    zr = wps.tile([P, H], F32, tag="zr")
                    nc.vector.reciprocal(zr[:], zs[:])
                    o1 = wps.tile([P, F1], F32, tag="o1")
                    nc.vector.tensor_mul(
                        out=o1[:].rearrange("p (c h) -> p c h", h=H),
                        in0=acc[:, :F1].rearrange("p (c h) -> p c h", h=H),
                        in1=zr[:].rearrange("p (o h) -> p o h", o=1).to_broadcast([P, C, H]),
                    )
                    rneg = wps.tile([P, F1], F32, tag="rneg")
                    nc.scalar.activation(rneg[:], o1[:], AF.Relu, scale=-1.0)
                    em = wps.tile([P, F1], F32, tag="em")
                    nc.scalar.activation(em[:], rneg[:], AF.Exp, scale=-1.0)
                    mx = wps.tile([P, F1], F32, tag="mx")
                    nc.scalar.activation(mx[:], o1[:], AF.Relu)
                    h1 = wps.tile([P, F1], BF16, tag="h1")
                    nc.vector.scalar_tensor_tensor(
                        out=h1[:], in0=em[:], scalar=-1.0, in1=mx[:],
                        op0=mybir.AluOpType.add, op1=mybir.AluOpType.add,
                    )
                    h1Tw = wps.tile([P, KT2 * P], BF16, tag="h1Tw")
                    for fc in range(KT2):
                        tp = psT.tile([P, P], BF16, tag="tp")
                        nc.tensor.transpose(tp[:], h1[:, fc * P:(fc + 1) * P], ident[:])
                        nc.scalar.activation(h1Tw[:, fc * P:(fc + 1) * P], tp[:], AF.Copy)
                    p2 = psC.tile([P, OCE], F32, tag="p2")
                    for k in range(KT2):
                        nc.tensor.matmul(
                            p2[:],
                            lhsT=h1Tw[:, k * P:(k + 1) * P],
                            rhs=w2sb[:, k * OCE:(k + 1) * OCE],
                            start=(k == 0), stop=(k == KT2 - 1),
                        )
                    # build the padded fp8 L2 gather row for this window:
                    # [h2(40) | 1.0 | pad | a_s2 x42 | pad]
                    nc.scalar.activation(
                        h2all[:, w * PC:w * PC + OUT_C], p2[:, :OUT_C], AF.Copy)
                    nc.vector.memset(h2all[:, w * PC + OUT_C:w * PC + OUT_C + 1], 1.0)
                    as2sb = wps.tile([P, 1], F32, tag="as2sb")
                    nc.vector.tensor_copy(out=as2sb[:], in_=p2[:, OUT_C:OUT_C + 1])
                    nc.scalar.activation(
                        h2all[:, w * PC + AS0:w * PC + AS1], zeros42[:],
                        AF.Identity, bias=as2sb[:])
                    nc.vector.tensor_copy(out=ad2w[:, w:w + 1], in_=p2[:, OCE - 1:OCE])
                    # layer-2 a_d per edge, computed now while stw tile is hot
                    for tl in range(TPW):
                        nc.tensor.matmul(
                            adew[:, TPW * H + tl:TPW * H + tl + 1],
                            lhsT=stq[:, tb + tl * P:tb + (tl + 1) * P],
                            rhs=ad2w[:, w:w + 1],
                            start=True, stop=True,
                        )
                    nc.scalar.activation(
                        adew2all[:, w * TPW:(w + 1) * TPW], adew[:, TPW * H:], AF.Copy)
                    # ship AllGather chunk inputs while MP1 continues
                    if w % CHW == CHW - 1:
                        q = w // CHW
                        nc.sync.dma_start(
                            out=h2loc_d[q * CR:(q + 1) * CR, :]
                                .rearrange("(t p) c -> p t c", p=P),
                            in_=h2all[:, q * CHW * PC:(q + 1) * CHW * PC]
                                .rearrange("p (t c) -> p t c", c=PC),
                        )
                # tail AllGather chunk fires right after its store
                q = W // CHW - 1
                nc.gpsimd.collective_compute(
                    "AllGather", mybir.AluOpType.bypass, replica_groups=rg,
                    ins=[h2loc_d[q * CR:(q + 1) * CR, :]],
                    outs=[h2pkc_d[q * CR * NCORES:(q + 1) * CR * NCORES, :]],
                )

            stwp_cm.__exit__(None, None, None)   # stw quarters dead
            swall_cm = tc.tile_pool(name="swall", bufs=1)
            swallp = swall_cm.__enter__()
            swall = swallp.tile([P, T * P], BF16)
            for q in range(4):
                nc.sync.dma_start(out=swall[:, q * QS:(q + 1) * QS],
                                  in_=swin_d[:, q * QS:(q + 1) * QS])
            # expand compact 96B AllGather rows to 256B-padded gather rows
            # (sub-chunked so chunk-A expands overlap the tail AllGather)
            XR = 2 * P * NCORES // P  # rows per partition per expand (2 windows)
            with tc.tile_pool(name="xp", bufs=2) as xp:
                for x0 in range(0, W, 2):
                    r0, r1 = x0 * P * NCORES, (x0 + 2) * P * NCORES
                    e96 = xp.tile([P, XR * PC], FP8, tag="e96")
                    nc.gpsimd.dma_start(
                        out=e96[:].rearrange("p (t c) -> p t c", c=PC),
                        in_=h2pkc_d[r0:r1, :].rearrange("(p t) c -> p t c", p=P))
                    e256 = xp.tile([P, XR * PR], FP8, tag="e256")
                    nc.vector.tensor_copy(
                        out=e256[:].rearrange("p (t c) -> p t c", c=PR)[:, :, :PC],
                        in_=e96[:].rearrange("p (t c) -> p t c", c=PC))
                    nc.sync.dma_start(
                        out=h2pk_d[r0:r1, :].rearrange("(p t) c -> p t c", p=P),
                        in_=e256[:].rearrange("p (t c) -> p t c", c=PR))

            # ---------------- Phase D: layer-2 message passing ----------------
            with (
                tc.tile_pool(name="gp2", bufs=10) as gp2,
                tc.tile_pool(name="gbp", bufs=10) as gbp,
                tc.tile_pool(name="psAcc2", bufs=2, space="PSUM") as psAcc2,
                tc.tile_pool(name="wp2", bufs=2) as wp2,
                tc.tile_pool(name="wps2", bufs=3) as wps2,
                tc.tile_pool(name="outp", bufs=2) as outp,
            ):
                chunk2 = {}

                def get_chunk2(cc):
                    if cc in chunk2:
                        return chunk2[cc]
                    g2 = gp2.tile([P, GC * PR], FP8, tag="gath2")
                    nc.gpsimd.dma_gather(
                        out_ap=g2[:].rearrange("p (t f) -> p t f", f=PR),
                        in_ap=h2pk_d.ap(),
                        idxs_ap=idx2sb[:, cc * IW:(cc + 1) * IW],
                        num_idxs=NI, num_idxs_reg=NI, elem_size=PR,
                        queue_num=cc % NSQ,
                    )
                    gb = gbp.tile([P, GC * CW], BF16, tag="gbf")
                    nc.scalar.activation(
                        gb[:].rearrange("p (t f) -> p t f", f=CW),
                        g2[:].rearrange("p (t f) -> p t f", f=PR)[:, :, :CW], AF.Copy)
                    chunk2[cc] = gb
                    return gb

                def segments2(w):
                    segs = []
                    t0, t1 = w * TPW, (w + 1) * TPW
                    t = t0
                    while t < t1:
                        cc = t // GC
                        s0 = t - cc * GC
                        s1 = min(GC, t1 - cc * GC)
                        segs.append((cc, s0, s1, t - t0))
                        t = cc * GC + s1
                    return segs

                for w in range(W):
                    swin2 = swall[:, w * TPW * P:(w + 1) * TPW * P]
                    segs = segments2(w)
                    for (cc, s0, s1, tl0) in segs:
                        get_chunk2(cc)
                    # packed 42-wide: esc = a_s2(rep) + a_d2(bcast); then
                    # Prelu/Exp give alpha replicated x42 with no picks
                    esc2w = wps2.tile([P, TPW * W42], BF16, tag="esc2w")
                    for (cc, s0, s1, tl0) in segs:
                        gb = chunk2[cc]
                        nt = s1 - s0
                        nc.vector.tensor_add(
                            out=esc2w[:].rearrange("p (t f) -> p t f", f=W42)[:, tl0:tl0 + nt, :],
                            in0=gb[:].rearrange("p (t f) -> p t f", f=CW)[:, s0:s1, AS0:AS1],
                            in1=adew2all[:, w * TPW + tl0:w * TPW + tl0 + nt]
                                .rearrange("p (t o) -> p t o", o=1)
                                .to_broadcast([P, nt, W42]),
                        )
                    lr2w = wps2.tile([P, TPW * W42], BF16, tag="lr2w")
                    nc.scalar.activation(lr2w[:], esc2w[:], AF.Prelu, alpha=NEG)
                    exp2w = wps2.tile([P, TPW * W42], BF16, tag="exp2w")
                    nc.scalar.activation(exp2w[:], lr2w[:], AF.Exp)
                    # rhs = [h2 | 1] * alpha, all packed 42-wide
                    rhs2 = wp2.tile([P, TPW * W42], BF16, tag="rhs2")
                    for (cc, s0, s1, tl0) in segs:
                        gb = chunk2[cc]
                        nt = s1 - s0
                        nc.vector.tensor_mul(
                            out=rhs2[:].rearrange("p (t f) -> p t f", f=W42)[:, tl0:tl0 + nt, :],
                            in0=gb[:].rearrange("p (t f) -> p t f", f=CW)[:, s0:s1, :W42],
                            in1=exp2w[:].rearrange("p (t f) -> p t f", f=W42)[:, tl0:tl0 + nt, :],
                        )
                    acc2 = psAcc2.tile([P, OC1], F32, tag="acc2")
                    for tl in range(TPW):
                        nc.tensor.matmul(
                            acc2[:], lhsT=swin2[:, tl * P:(tl + 1) * P],
                            rhs=rhs2[:, tl * W42:tl * W42 + OC1],
                            start=(tl == 0), stop=(tl == TPW - 1),
                        )
                    zs2 = wps2.tile([P, 1], F32, tag="zs2")
                    nc.vector.tensor_scalar_add(out=zs2[:], in0=acc2[:, OUT_C:OC1], scalar1=1e-16)
                    zr2 = wps2.tile([P, 1], F32, tag="zr2")
                    nc.vector.reciprocal(zr2[:], zs2[:])
                    nc.vector.tensor_mul(
                        out=o2all[:, w * OUT_C:(w + 1) * OUT_C], in0=acc2[:, :OUT_C],
                        in1=zr2[:].to_broadcast([P, OUT_C]),
                    )
                    nc.vector.tensor_reduce(
                        out=mnall[:, w:w + 1], in_=o2all[:, w * OUT_C:(w + 1) * OUT_C],
                        axis=mybir.AxisListType.X,
                        op=mybir.AluOpType.max, negate=True,
                    )
                    ex = wps2.tile([P, OUT_C], F32, tag="ex")
                    nc.scalar.activation(
                        ex[:], o2all[:, w * OUT_C:(w + 1) * OUT_C], AF.Exp,
                        bias=mnall[:, w:w + 1], accum_out=ssall[:, w:w + 1],
                    )

                # single Ln pass over all windows, then per-window bias add
                lnall = wps2.tile([P, W], F32, tag="lnall")
                nc.scalar.activation(lnall[:], ssall[:], AF.Ln)
                comb = wps2.tile([P, W], F32, tag="comb")
                nc.vector.tensor_sub(out=comb[:], in0=mnall[:], in1=lnall[:])
                for w in range(W):
                    if w % 4 == 0:
                        fin4 = outp.tile([P, 4 * OUT_C], F32, tag="fin4")
                    nc.scalar.activation(
                        fin4[:, (w % 4) * OUT_C:(w % 4 + 1) * OUT_C],
                        o2all[:, w * OUT_C:(w + 1) * OUT_C],
                        AF.Identity, bias=comb[:, w:w + 1])
                    if w % 4 == 3:
                        g4 = w // 4
                        nc.scalar.dma_start(
                            out=out_d[g4 * 4 * P:(g4 + 1) * 4 * P, :]
                                .rearrange("(t p) c -> p t c", p=P),
                            in_=fin4[:].rearrange("p (t c) -> p t c", c=OUT_C),
                        )
            swall_cm.__exit__(None, None, None)

    nc.compile()
    return nc


def _prepare(x, edge_index, w1, att_src1, att_dst1, b1, w2, att_src2, att_dst2, b2):
    x = np.asarray(x, dtype=np.float32)
    edge_index = np.asarray(edge_index)
    w1 = np.asarray(w1, dtype=np.float32)
    att_src1 = np.asarray(att_src1, dtype=np.float32)
    att_dst1 = np.asarray(att_dst1, dtype=np.float32)
    b1 = np.asarray(b1, dtype=np.float32)
    w2 = np.asarray(w2, dtype=np.float32)
    att_src2 = np.asarray(att_src2, dtype=np.float32)
    att_dst2 = np.asarray(att_dst2, dtype=np.float32)
    b2 = np.asarray(b2, dtype=np.float32)
    assert not np.any(b1) and not np.any(b2), "nonzero bias unsupported"

    N, IN_C = x.shape
    H, C = att_src1.shape
    F1 = H * C
    OUT_C = w2.shape[1]
    assert N % NCORES == 0
    NPC = N // NCORES
    W = (NPC + P - 1) // P
    NPAD = W * P
    NROWS = NCORES * NPAD
    assert NROWS < 32768
    assert W % CHW == 0

    # ---- edges: append self-loops, sort by destination ----
    src = np.concatenate([edge_index[0].astype(np.int64), np.arange(N, dtype=np.int64)])
    dst = np.concatenate([edge_index[1].astype(np.int64), np.arange(N, dtype=np.int64)])
    order = np.argsort(dst, kind="stable")
    src, dst = src[order], dst[order]

    core_of = dst // NPC
    bounds = np.searchsorted(dst, np.arange(NCORES + 1) * NPC)
    win_of = (dst - core_of * NPC) // P

    counts = np.zeros((NCORES, W), np.int64)
    for cidx in range(NCORES):
        w_arr = win_of[bounds[cidx]:bounds[cidx + 1]]
        counts[cidx] = np.bincount(w_arr, minlength=W)
    TPW = max(1, int(np.ceil(counts.max() / P)))
    T = W * TPW
    TG = ((T + GC - 1) // GC) * GC

    blocked = (src // NPC) * NPAD + (src % NPC)     # global padded row of src

    # layer-2 rows live in AllGather-chunked layout:
    # node (c, r): q = r // (CHW*P); row = q*CHW*P*NCORES + c*CHW*P + (r - q*CHW*P)
    CR = CHW * P
    src_c = src // NPC
    src_r = src % NPC
    src_q = src_r // CR
    blocked2 = src_q * CR * NCORES + src_c * CR + (src_r - src_q * CR)

    NI = GC * P
    IW = NI // 16
    NCH = TG // GC

    def pack_idx(ids):
        """gather-index layout: chunk cc's idxs at cols [cc*IW,(cc+1)*IW)."""
        idx16 = np.zeros((16, NCH * IW), np.int16)
        gpos = np.arange(TG * P)
        cc, ii = gpos // NI, gpos % NI
        full = np.zeros(TG * P, np.int16)
        full[:len(ids)] = ids
        idx16[ii % 16, cc * IW + ii // 16] = full
        return np.tile(idx16, (8, 1))

    in_maps = []
    xTf = np.zeros((IN_C, NROWS), np.float32)
    xTf = xTf.reshape(IN_C, NCORES, NPAD)
    xTf[:, :, :NPC] = x.T.reshape(IN_C, NCORES, NPC)
    xTf = xTf.reshape(IN_C, NROWS)
    xTf_bf = _to_bf(xTf)

    # head-interleaved permutation: new col c*H+h <- old col h*C+c
    f_old = np.arange(F1)
    h_idx, c_idx = f_old // C, f_old % C
    f_new = c_idx * H + h_idx
    perm = np.empty(F1, np.int64)
    perm[f_new] = f_old          # perm[new] = old

    Asrc = np.zeros((F1, H), np.float32)
    Adst = np.zeros((F1, H), np.float32)
    for h in range(H):
        Asrc[h * C:(h + 1) * C, h] = att_src1[h]
        Adst[h * C:(h + 1) * C, h] = att_dst1[h]
    w1P = w1[:, perm]
    w1e = np.concatenate([w1P, w1 @ Asrc, w1 @ Adst], axis=1)
    w2P = w2[perm, :]
    w2e = np.concatenate([w2P, w2P @ att_src2.T, w2P @ att_dst2.T], axis=1)
    w1e_bf = _to_bf(w1e)
    w2e_bf = _to_bf(w2e)

    nc = _build_program(IN_C, F1, H, C, OUT_C, NPAD, T, TPW, W, TG, NROWS)

    for cidx in range(NCORES):
        ids_g = np.zeros(T * P, np.int64)         # L1: global padded row per slot
        ids_g2 = np.zeros(T * P, np.int64)        # L2: chunked-AllGather row per slot
        dloc = np.full(T * P, 255, np.int64)      # pad -> no one-hot match
        s_c = blocked[bounds[cidx]:bounds[cidx + 1]]
        s2_c = blocked2[bounds[cidx]:bounds[cidx + 1]]
        w_c = win_of[bounds[cidx]:bounds[cidx + 1]]
        d_c = dst[bounds[cidx]:bounds[cidx + 1]] - cidx * NPC
        wb = np.searchsorted(w_c, np.arange(W + 1))
        for w in range(W):
            n = wb[w + 1] - wb[w]
            base = w * TPW * P
            ids_g[base:base + n] = s_c[wb[w]:wb[w + 1]]
            ids_g2[base:base + n] = s2_c[wb[w]:wb[w + 1]]
            dloc[base:base + n] = d_c[wb[w]:wb[w + 1]] - w * P
        ids_rot = (ids_g - cidx * NPAD) % NROWS   # L1 rows are core-rotated

        # one-hot scatter/gather matrices
        M = np.zeros((T * P, P), np.float32)
        real = dloc < P
        M[np.nonzero(real)[0], dloc[real]] = 1.0
        M3 = M.reshape(T, P, P)
        swin = _to_bf(np.ascontiguousarray(M3.transpose(1, 0, 2)).reshape(P, T * P))
        stw_m = _to_bf(np.ascontiguousarray(M3.transpose(2, 0, 1)).reshape(P, T * P))

        in_maps.append({
            "xT": np.ascontiguousarray(np.roll(xTf_bf, -cidx * NPAD, axis=1)) if cidx else xTf_bf,
            "w1e": w1e_bf,
            "w2e": w2e_bf,
            "idx1": pack_idx(ids_rot.astype(np.int16)),
            "idx2": pack_idx(ids_g2.astype(np.int16)),
            "swin": swin,
            "stw": stw_m,
        })
    return nc, in_maps, NPC


def kernel(_trace=False, **inputs):
    nc, in_maps, NPC = _prepare(**inputs)
    res = run_bass_kernel_spmd(nc, in_maps, core_ids=list(range(NCORES)), trace=_trace)
    out = np.concatenate([res.results[cidx]["out"][:NPC] for cidx in range(NCORES)], axis=0)
    kernel.last_exec_time_ns = res.exec_time_ns
    kernel.last_res = res
    return out.astype(np.float32)
